# revision 12
# baseline (speedup 1.0000x reference)
"""AdaConv kernel for 8 TRN2 NeuronCores — data-parallel over batch.

Two-stage stencil formulation. Math identical to the reference after
collapsing the per-sample grouped convs:
    D[n,g,h,w] = sum_{j,kh,kw} d[n,j,kh,kw] * xpad[n,8g+j,h+kh,w+kw]
    out = leaky(S[n]*D[n,c//8] + bias[n,c]) * (x - mean)/std

Per core (2 samples, 8 tiles of 128 channels; x stored bf16, pitch-68 rows):
  stage 1 (PE): 3 accumulating matmuls (kw taps via rhs column offsets),
      M = 96 = (kh,g) pairs 32-aligned -> P2[(kh,g), p] in PSUM.
  fold: PSUM->SBUF copy drops the seam cols (dense 64-pitch bf16), then two
      SBUF->SBUF DMAs shift the kh=1,2 slabs by kh*64 so taps align.
  stage 2 (PE): ONE K=48 matmul per 7-row slot -> D replicated over the 8
      channels of each group, directly in [128, px] dense layout.
  ScalarE evicts with fused leaky(S*rstd*D + bias*rstd) (valid: r>0).
  Stats on DVE: bn_stats/bn_aggr (bf16 x); rstd via Newton rsqrt on DVE
      (no ScalarE Sqrt -> no activation-table thrash).
  Final: DVE tensor_scalar xn = x - mean, tensor_tensor out = xn * predn.
"""

import numpy as np
import ml_dtypes
from contextlib import ExitStack

import concourse.bass as bass
import concourse.tile as tile
from concourse import bacc, mybir
from concourse.bass_utils import run_bass_kernel_spmd

F32 = mybir.dt.float32
I32 = mybir.dt.int32
BF16 = mybir.dt.bfloat16
AF = mybir.ActivationFunctionType
ALU = mybir.AluOpType
AX = mybir.AxisListType

N_CORES = 8
NSAMP = 2           # samples per core
CH = 512
H = W = 64
PW = 68             # row pitch (junk col 0, padded cols 1..66, junk col 67)
NR = 66             # padded rows
EXT = PW * NR       # 4488 flat extent
XAL = EXT + 8       # 4496 allocated (stencil over-read + even)
DE = 64 * NR        # 4224 dense P2 extent
RSQRT_MAGIC = np.int32(0x5F3759DF).view(np.float32).item()

LAST_RESULTS = None  # BassKernelResults of the most recent run (for test.py)
_CACHE = {}


def _build():
    nc = bacc.Bacc("TRN2", target_bir_lowering=False, debug=False)

    x16_d = nc.dram_tensor("x16", [8, 128, XAL], BF16, kind="ExternalInput")
    style_d = nc.dram_tensor("style", [128, NSAMP, 4, 16], F32, kind="ExternalInput")
    dwT_d = nc.dram_tensor("dwT", [128, 2, 2, 4, 8], F32, kind="ExternalInput")
    dwb_d = nc.dram_tensor("dwb", [8, 1], F32, kind="ExternalInput")
    pbT_d = nc.dram_tensor("pbT", [128, 4, 512], F32, kind="ExternalInput")
    pbb_d = nc.dram_tensor("pbb", [128, 4], F32, kind="ExternalInput")
    pkwT_d = nc.dram_tensor("pkwT", [128, 4, 8], F32, kind="ExternalInput")
    pkb_d = nc.dram_tensor("pkb", [1, 8], F32, kind="ExternalInput")
    mask32_d = nc.dram_tensor("mask32", [128, 32], BF16, kind="ExternalInput")
    w2_d = nc.dram_tensor("w2", [48, 128], BF16, kind="ExternalInput")
    repl8_d = nc.dram_tensor("repl8", [8, 128], F32, kind="ExternalInput")
    out_d = nc.dram_tensor("out", [8, 128, H * W], BF16, kind="ExternalOutput")

    with tile.TileContext(nc) as tc, ExitStack() as ctx:
        const = ctx.enter_context(tc.tile_pool(name="const", bufs=1))
        small = ctx.enter_context(tc.tile_pool(name="small", bufs=1))
        x16p = ctx.enter_context(tc.tile_pool(name="x16", bufs=8))
        p2p = ctx.enter_context(tc.tile_pool(name="p2", bufs=2))
        statp = ctx.enter_context(tc.tile_pool(name="stat", bufs=2))
        xnp = ctx.enter_context(tc.tile_pool(name="xn", bufs=2))
        prednp = ctx.enter_context(tc.tile_pool(name="pred", bufs=2))
        outp = ctx.enter_context(tc.tile_pool(name="outp", bufs=2))
        psumA = ctx.enter_context(
            tc.tile_pool(name="psumA", bufs=2, space="PSUM"))
        psumB = ctx.enter_context(
            tc.tile_pool(name="psumB", bufs=2, space="PSUM"))

        # ---- params (small, first on the sync queue) ----
        style_sb = const.tile([128, NSAMP, 4, 16], F32)
        nc.sync.dma_start(style_sb[:], style_d[:])
        dwT_sb = const.tile([128, 2, 2, 4, 8], F32)
        nc.sync.dma_start(dwT_sb[:], dwT_d[:])
        dwb_sb = const.tile([8, 1], F32)
        nc.sync.dma_start(dwb_sb[:], dwb_d[:])
        repl8_sb = const.tile([8, 128], F32)
        nc.sync.dma_start(repl8_sb[:], repl8_d[:])
        mask32_sb = const.tile([128, 32], BF16)
        nc.sync.dma_start(mask32_sb[:], mask32_d[:])
        w2_sb = const.tile([48, 128], BF16)
        nc.sync.dma_start(w2_sb[:], w2_d[:])
        pkb_sb = const.tile([1, 8], F32)
        nc.scalar.dma_start(pkb_sb[:], pkb_d[:])
        pbb_sb = const.tile([128, 4], F32)
        nc.scalar.dma_start(pbb_sb[:], pbb_d[:])
        pbT_sb = const.tile([128, 4, 512], F32)
        nc.scalar.dma_start(pbT_sb[:], pbT_d[:])
        pkwT_sb = const.tile([128, 4, 8], F32)
        nc.scalar.dma_start(pkwT_sb[:], pkwT_d[:])

        # content loads for the first tiles (prefetch window of 3)
        x16s = []
        for _ in range(8):
            x16 = x16p.tile([128, XAL], BF16, tag="x16")
            x16s.append(x16)

        def emit_xin(ts):
            for c in range(4):
                lo, hi = c * 1124, (c + 1) * 1124
                nc.sync.dma_start(x16s[ts][:, lo:hi], x16_d[ts][:, lo:hi])

        for ts in range(3):
            emit_xin(ts)

        # ---- prologue: kernel-predictor math (tiny, f32) ----
        W1_sb = const.tile([128, NSAMP, 3, 96], BF16)     # stage-1 weights
        bias_sb = const.tile([128, 4, NSAMP], F32)        # per-channel bias
        Sb_sb = const.tile([128, NSAMP], F32)             # S[n] on 128 parts
        d_sb = small.tile([8, NSAMP, 9], F32)
        dcol_sb = small.tile([128, NSAMP, 9], F32)
        ssum_sb = small.tile([128, 4, NSAMP], F32)
        pkwsum_sb = small.tile([128, 4], F32)
        pkbsum_sb = small.tile([1, 1], F32)
        S_sb = small.tile([1, NSAMP], F32)
        magic_sb = const.tile([128, 1], F32)
        nc.vector.memset(magic_sb[:], RSQRT_MAGIC)
        nc.vector.memset(W1_sb[:], 0.0)

        nc.vector.tensor_reduce(pkbsum_sb[:], pkb_sb[:], axis=AX.X, op=ALU.add)
        for kt in range(4):
            nc.vector.tensor_reduce(
                pkwsum_sb[:, kt:kt + 1], pkwT_sb[:, kt, :], axis=AX.X, op=ALU.add)

        for s in range(NSAMP):
            # d = leaky(conv2x2(style, dw_w) + dw_b):  16 accumulating matmuls
            psA0 = psumA.tile([128, 1024], F32, tag="psA")
            ps_d = psA0[0:8, 0:9]
            i = 0
            for ky in range(2):
                for kx in range(2):
                    for kt in range(4):
                        rhs = style_sb[:, s, kt, :].rearrange(
                            "p (y x) -> p y x", x=4)[:, ky:ky + 3, kx:kx + 3]
                        nc.tensor.matmul(
                            ps_d, dwT_sb[:, ky, kx, kt, :], rhs,
                            start=(i == 0), stop=(i == 15))
                        i += 1
            nc.scalar.activation(
                d_sb[:, s, :], ps_d, AF.Lrelu, bias=dwb_sb[:], alpha=0.01)

            # replicate d over channels: dcol[c,t] = d[c%8,t]
            psA1 = psumA.tile([128, 1024], F32, tag="psA")
            ps_dc = psA1[:, 0:9]
            nc.tensor.matmul(ps_dc, repl8_sb[:], d_sb[:, s, :])
            nc.vector.tensor_copy(dcol_sb[:, s, :], ps_dc)

            # stage-1 weights W1[kw][ch, kh*32+g] = d[ch%8, kh, kw]*(g==ch//8)
            for kh in range(3):
                for kw in range(3):
                    nc.vector.tensor_scalar(
                        W1_sb[:, s, kw, kh * 32: kh * 32 + 32], mask32_sb[:],
                        dcol_sb[:, s, 3 * kh + kw: 3 * kh + kw + 1], None,
                        ALU.mult)

            # style spatial sums (s_d * 16)
            for kt in range(4):
                nc.vector.tensor_reduce(
                    ssum_sb[:, kt, s:s + 1], style_sb[:, s, kt, :],
                    axis=AX.X, op=ALU.add)

        # bias[c] = s_d @ pb_w[c] + pb_b[c]   (both samples batched)
        for mt in range(4):
            psB0 = psumB.tile([128, 1024], F32, tag="psB")
            ps_b = psB0[:, 0:NSAMP]
            for kt in range(4):
                nc.tensor.matmul(
                    ps_b, pbT_sb[:, kt, mt * 128:(mt + 1) * 128],
                    ssum_sb[:, kt, :], start=(kt == 0), stop=(kt == 3))
            nc.vector.tensor_scalar(
                bias_sb[:, mt, :], ps_b, 1.0 / 16.0,
                pbb_sb[:, mt:mt + 1], ALU.mult, ALU.add)

        # S = s_d @ pkw_sum + sum(pk_b)
        psB1 = psumB.tile([128, 1024], F32, tag="psB")
        ps_S = psB1[0:1, 0:NSAMP]
        for kt in range(4):
            nc.tensor.matmul(
                ps_S, pkwsum_sb[:, kt:kt + 1], ssum_sb[:, kt, :],
                start=(kt == 0), stop=(kt == 3))
        nc.vector.tensor_scalar(
            S_sb[:], ps_S, 1.0 / 16.0, pkbsum_sb[:], ALU.mult, ALU.add)
        nc.gpsimd.partition_broadcast(Sb_sb[:], S_sb[:])

        # ---- per-tile state ----
        mv_all = small.tile([128, 8, 2], F32)        # (mean, var) per tile
        v_all = small.tile([128, 8], F32)            # var + eps
        y_all = small.tile([128, 8], F32)            # rsqrt iterate
        t_all = small.tile([128, 8], F32)
        scaleS_all = small.tile([128, 8], F32)       # S * rstd
        biasS_all = small.tile([128, 8], F32)        # bias * rstd

        def bn_stats_raw(out, in_):
            # bass's bn_stats wrapper mis-asserts the out shape for 3D
            # inputs; the HW op always writes 6 elements/partition.
            eng = nc.vector
            return eng.add_instruction(mybir.InstBNStats(
                name=eng.bass.get_next_instruction_name(),
                ins=[eng.lower_ap(in_)], outs=[eng.lower_ap(out)]))

        def emit_stats(ts):
            xr = x16s[ts][:, :EXT].rearrange("p (r w) -> p r w", w=PW)
            st = statp.tile([128, 8, 6], F32, tag="bn")
            for i in range(8):
                bn_stats_raw(st[:, i, :], xr[:, 1 + 8 * i: 9 + 8 * i, 2:66])
            nc.vector.bn_aggr(mv_all[:, ts, :], st[:])
            # xn = x - mean (bf16, dense) on the otherwise-idle GPSIMD
            xn = xnp.tile([128, H * W], BF16, tag="xn")
            nc.gpsimd.tensor_scalar(
                xn[:].rearrange("p (r w) -> p r w", w=64),
                xr[:, 1:65, 2:66], mv_all[:, ts, 0:1], None, ALU.subtract)
            return xn

        def emit_finalize(p):
            # pair-batched: tiles 2p, 2p+1
            sl = slice(2 * p, 2 * p + 2)
            s = (2 * p) // 4
            # v = var*4096/4095 + eps
            nc.vector.tensor_scalar(
                v_all[:, sl], mv_all[:, sl, 1], 4096.0 / 4095.0, 1e-5,
                ALU.mult, ALU.add)
            # Newton rsqrt: y0 from the bit trick, then 2 iterations
            nc.vector.tensor_scalar(
                t_all[:, sl].bitcast(I32), v_all[:, sl].bitcast(I32), 1,
                None, ALU.arith_shift_right)
            nc.vector.tensor_tensor(
                y_all[:, sl].bitcast(I32),
                magic_sb[:].bitcast(I32).to_broadcast([128, 2]),
                t_all[:, sl].bitcast(I32), ALU.subtract)
            y, t, v = y_all[:, sl], t_all[:, sl], v_all[:, sl]
            for _ in range(2):
                nc.vector.tensor_tensor(t, y, y, ALU.mult)
                nc.vector.tensor_tensor(t, t, v, ALU.mult)
                nc.vector.tensor_scalar(t, t, -0.5, 1.5, ALU.mult, ALU.add)
                nc.vector.tensor_tensor(y, y, t, ALU.mult)
            nc.vector.tensor_scalar(
                scaleS_all[:, sl], y, Sb_sb[:, s:s + 1], None, ALU.mult)
            kt0 = (2 * p) % 4
            nc.vector.tensor_tensor(
                biasS_all[:, sl], y, bias_sb[:, kt0:kt0 + 2, s], ALU.mult)

        def emit_stage1(ts):
            s = ts // 4
            x16 = x16s[ts]
            p2 = p2p.tile([96, DE], BF16, tag="p2")
            # row-aligned 7-row slots; 2 slots per 2-bank psum tile
            r0 = 0
            ti = 0
            while r0 < NR:
                psA = psumA.tile([128, 1024], F32, tag="psA")
                rows = []
                for sub in (0, 512):
                    nr = min(7, NR - r0 - sum(rows))
                    if nr <= 0:
                        break
                    rows.append(nr)
                for si, nr in enumerate(rows):
                    rr = r0 + (rows[0] if si else 0)
                    lo = rr * PW
                    cw = nr * PW
                    for kw in range(3):
                        nc.tensor.matmul(
                            psA[0:96, si * 512: si * 512 + cw],
                            W1_sb[:, s, kw, :],
                            x16[:, lo + kw: lo + kw + cw],
                            start=(kw == 0), stop=(kw == 2))
                nrt = sum(rows)
                # seam-dropping copy: [96, slot, row, 68 -> 64] -> dense
                if len(rows) == 2 and rows[0] == rows[1]:
                    src = psA[0:96, :].rearrange(
                        "p (u q) -> p u q", q=512)[:, :, :rows[0] * PW] \
                        .rearrange("p u (r w) -> p u r w", w=PW)[:, :, :, 1:65]
                    dst = p2[:, r0 * 64: (r0 + nrt) * 64]
                    cop = nc.scalar.copy if ts % 2 == 0 else nc.vector.tensor_copy
                    cop(dst, src)
                else:
                    off = 0
                    for si, nr in enumerate(rows):
                        src = psA[0:96, si * 512: si * 512 + nr * PW].rearrange(
                            "p (r w) -> p r w", w=PW)[:, :, 1:65]
                        dst = p2[:, (r0 + off) * 64: (r0 + off + nr) * 64]
                        cop = (nc.scalar.copy if ts % 2 == 0
                               else nc.vector.tensor_copy)
                        cop(dst, src)
                        off += nr
                r0 += nrt
                ti += 1
            # pre-apply kh row shifts (dense pitch 64)
            nc.gpsimd.dma_start(p2[16:32, 0:DE - 64], p2[32:48, 64:DE])
            nc.gpsimd.dma_start(p2[32:48, 0:DE - 128], p2[64:80, 128:DE])
            return p2

        def emit_stage2(ts, p2, xn):
            predn = prednp.tile([128, H * W], BF16, tag="pred")
            out_sb = outp.tile([128, H * W], BF16, tag="out")
            for half in range(2):
                for rt in (0, 16):
                    r0 = half * 32 + rt
                    psB = psumB.tile([128, 1024], F32, tag="psB")
                    for sub in (0, 512):
                        rr = r0 + (8 if sub else 0)
                        nc.tensor.matmul(
                            psB[:, sub:sub + 512], w2_sb[:],
                            p2[0:48, rr * 64: (rr + 8) * 64],
                            start=True, stop=True)
                    nc.scalar.activation(
                        predn[:, r0 * 64: (r0 + 16) * 64], psB[:, :],
                        AF.Lrelu, bias=biasS_all[:, ts:ts + 1],
                        scale=scaleS_all[:, ts:ts + 1], alpha=0.01)
                lo, hi = half * 2048, (half + 1) * 2048
                nc.vector.tensor_tensor(
                    out_sb[:, lo:hi], xn[:, lo:hi], predn[:, lo:hi], ALU.mult)
                nc.sync.dma_start(out_d[ts][:, lo:hi], out_sb[:, lo:hi])

        # ---- software-pipelined main loop: stage1(k+1) ahead of stage2(k) ----
        xn0 = emit_stats(0)
        p2_prev, xn_prev = emit_stage1(0), xn0
        for k in range(8):
            if k + 1 < 8:
                if k + 3 < 8:
                    emit_xin(k + 3)
                xn_next = emit_stats(k + 1)
                if (k + 1) % 2 == 1:
                    emit_finalize((k + 1) // 2)
                p2_next = emit_stage1(k + 1)
            emit_stage2(k, p2_prev, xn_prev)
            if k + 1 < 8:
                p2_prev, xn_prev = p2_next, xn_next

    nc.compile()
    return nc


def _host_prep(style_encoding, content_in, dw_w, dw_b, pk_w, pk_b, pb_w, pb_b):
    """Shard + lay out inputs for the 8 cores (layout only, no math)."""
    f32 = np.float32
    bf = ml_dtypes.bfloat16
    common = {
        "dwT": np.ascontiguousarray(
            dw_w.reshape(8, 4, 128, 2, 2).transpose(2, 3, 4, 1, 0), f32),
        "dwb": np.ascontiguousarray(dw_b.reshape(8, 1), f32),
        "pbT": np.ascontiguousarray(
            pb_w.T.reshape(4, 128, 512).transpose(1, 0, 2), f32),
        "pbb": np.ascontiguousarray(pb_b.reshape(4, 128).T, f32),
        "pkwT": np.ascontiguousarray(
            pk_w.T.reshape(4, 128, 8).transpose(1, 0, 2), f32),
        "pkb": np.ascontiguousarray(pk_b.reshape(1, 8), f32),
    }
    ii = np.arange(128)
    common["mask32"] = (np.arange(32)[None, :] == (ii[:, None] // 8)).astype(bf)
    w2 = np.zeros((48, 128), bf)
    for kh in range(3):
        w2[kh * 16 + ii // 8, ii] = 1
    common["w2"] = w2
    common["repl8"] = (np.arange(8)[:, None] == (ii[None, :] % 8)).astype(f32)

    # padded pitch-68 bf16 content, all cores at once
    xp = np.pad(content_in, ((0, 0), (0, 0), (1, 1), (1, 1)), mode="reflect")
    buf = np.zeros((16, CH, NR, PW), f32)
    buf[:, :, :, 1:67] = xp
    xb = buf.reshape(16, 4, 128, EXT).astype(bf)

    in_maps = []
    for i in range(N_CORES):
        x16 = np.zeros((NSAMP, 4, 128, XAL), bf)
        x16[:, :, :, :EXT] = xb[NSAMP * i: NSAMP * (i + 1)]
        se = style_encoding[NSAMP * i: NSAMP * (i + 1)]
        in_maps.append({
            "x16": np.ascontiguousarray(x16.reshape(8, 128, XAL)),
            "style": np.ascontiguousarray(
                se.reshape(NSAMP, 4, 128, 16).transpose(2, 0, 1, 3), f32),
            **common,
        })
    return in_maps


def kernel(style_encoding, content_in, dw_w, dw_b, pk_w, pk_b, pb_w, pb_b):
    global LAST_RESULTS
    import os
    if "nc" not in _CACHE:
        _CACHE["nc"] = _build()
    nc = _CACHE["nc"]
    in_maps = _host_prep(style_encoding, content_in, dw_w, dw_b,
                         pk_w, pk_b, pb_w, pb_b)
    res = run_bass_kernel_spmd(
        nc, in_maps, core_ids=list(range(N_CORES)),
        trace=bool(os.environ.get("ADACONV_TRACE")))
    LAST_RESULTS = res
    outs = []
    for i in range(N_CORES):
        o = np.asarray(res.results[i]["out"]).astype(np.float32)
        outs.append(o.reshape(NSAMP, 4, 128, 64, 64).reshape(NSAMP, CH, 64, 64))
    return np.concatenate(outs, axis=0)


# revision 13
# speedup vs baseline: 3.9290x; 3.9290x over previous
"""AdaConv kernel for 8 TRN2 NeuronCores — data-parallel over batch.

Two-stage stencil formulation. Math identical to the reference after
collapsing the per-sample grouped convs:
    D[n,g,h,w] = sum_{j,kh,kw} d[n,j,kh,kw] * xpad[n,8g+j,h+kh,w+kw]
    out = leaky(S[n]*D[n,c//8] + bias[n,c]) * (x - mean)/std

Per core (2 samples, 8 tiles of 128 channels; x stored bf16, pitch-68 rows):
  stage 1 (PE): 3 accumulating matmuls (kw taps via rhs column offsets),
      M = 96 = (kh,g) pairs 32-aligned -> P2[(kh,g), p] in PSUM.
  fold: PSUM->SBUF copy drops the seam cols (dense 64-pitch bf16), then two
      SBUF->SBUF DMAs shift the kh=1,2 slabs by kh*64 so taps align.
  stage 2 (PE): ONE K=48 matmul per 7-row slot -> D replicated over the 8
      channels of each group, directly in [128, px] dense layout.
  ScalarE evicts with fused leaky(S*rstd*D + bias*rstd) (valid: r>0).
  Stats on DVE: bn_stats/bn_aggr (bf16 x); rstd via Newton rsqrt on DVE
      (no ScalarE Sqrt -> no activation-table thrash).
  Final: DVE tensor_scalar xn = x - mean, tensor_tensor out = xn * predn.
"""

import numpy as np
import ml_dtypes
from contextlib import ExitStack

import concourse.bass as bass
import concourse.tile as tile
from concourse import bacc, mybir
from concourse.bass_utils import run_bass_kernel_spmd

F32 = mybir.dt.float32
I32 = mybir.dt.int32
BF16 = mybir.dt.bfloat16
AF = mybir.ActivationFunctionType
ALU = mybir.AluOpType
AX = mybir.AxisListType

N_CORES = 8
NSAMP = 2           # samples per core
CH = 512
H = W = 64
PW = 68             # row pitch (junk col 0, padded cols 1..66, junk col 67)
NR = 66             # padded rows
EXT = PW * NR       # 4488 flat extent
XAL = EXT + 8       # 4496 allocated (stencil over-read + even)
DE = 64 * NR        # 4224 dense P2 extent
RSQRT_MAGIC = np.int32(0x5F3759DF).view(np.float32).item()

LAST_RESULTS = None  # BassKernelResults of the most recent run (for test.py)
_CACHE = {}


def _build():
    nc = bacc.Bacc("TRN2", target_bir_lowering=False, debug=False)

    x16_d = nc.dram_tensor("x16", [8, 128, XAL], BF16, kind="ExternalInput")
    style_d = nc.dram_tensor("style", [128, NSAMP, 4, 16], F32, kind="ExternalInput")
    dwT_d = nc.dram_tensor("dwT", [128, 2, 2, 4, 8], F32, kind="ExternalInput")
    dwb_d = nc.dram_tensor("dwb", [8, 1], F32, kind="ExternalInput")
    pbT_d = nc.dram_tensor("pbT", [128, 4, 512], F32, kind="ExternalInput")
    pbb_d = nc.dram_tensor("pbb", [128, 4], F32, kind="ExternalInput")
    pkwT_d = nc.dram_tensor("pkwT", [128, 4, 8], F32, kind="ExternalInput")
    pkb_d = nc.dram_tensor("pkb", [1, 8], F32, kind="ExternalInput")
    mask32_d = nc.dram_tensor("mask32", [128, 32], BF16, kind="ExternalInput")
    w2_d = nc.dram_tensor("w2", [48, 128], BF16, kind="ExternalInput")
    repl8_d = nc.dram_tensor("repl8", [8, 128], F32, kind="ExternalInput")
    out_d = nc.dram_tensor("out", [8, 128, H * W], BF16, kind="ExternalOutput")

    with tile.TileContext(nc) as tc, ExitStack() as ctx:
        const = ctx.enter_context(tc.tile_pool(name="const", bufs=1))
        small = ctx.enter_context(tc.tile_pool(name="small", bufs=1))
        x16p = ctx.enter_context(tc.tile_pool(name="x16", bufs=8))
        p2p = ctx.enter_context(tc.tile_pool(name="p2", bufs=2))
        statp = ctx.enter_context(tc.tile_pool(name="stat", bufs=2))
        xnp = ctx.enter_context(tc.tile_pool(name="xn", bufs=2))
        prednp = ctx.enter_context(tc.tile_pool(name="pred", bufs=2))
        outp = ctx.enter_context(tc.tile_pool(name="outp", bufs=2))
        psumA = ctx.enter_context(
            tc.tile_pool(name="psumA", bufs=2, space="PSUM"))
        psumB = ctx.enter_context(
            tc.tile_pool(name="psumB", bufs=2, space="PSUM"))

        # ---- params (small, first on the sync queue) ----
        style_sb = const.tile([128, NSAMP, 4, 16], F32)
        nc.sync.dma_start(style_sb[:], style_d[:])
        dwT_sb = const.tile([128, 2, 2, 4, 8], F32)
        nc.sync.dma_start(dwT_sb[:], dwT_d[:])
        dwb_sb = const.tile([8, 1], F32)
        nc.sync.dma_start(dwb_sb[:], dwb_d[:])
        repl8_sb = const.tile([8, 128], F32)
        nc.sync.dma_start(repl8_sb[:], repl8_d[:])
        mask32_sb = const.tile([128, 32], BF16)
        nc.sync.dma_start(mask32_sb[:], mask32_d[:])
        w2_sb = const.tile([48, 128], BF16)
        nc.sync.dma_start(w2_sb[:], w2_d[:])
        pkb_sb = const.tile([1, 8], F32)
        nc.scalar.dma_start(pkb_sb[:], pkb_d[:])
        pbb_sb = const.tile([128, 4], F32)
        nc.scalar.dma_start(pbb_sb[:], pbb_d[:])
        pbT_sb = const.tile([128, 4, 512], F32)
        nc.scalar.dma_start(pbT_sb[:], pbT_d[:])
        pkwT_sb = const.tile([128, 4, 8], F32)
        nc.scalar.dma_start(pkwT_sb[:], pkwT_d[:])

        # content loads for the first tiles (prefetch window of 3)
        x16s = []
        for _ in range(8):
            x16 = x16p.tile([128, XAL], BF16, tag="x16")
            x16s.append(x16)

        def emit_xin(ts):
            for c in range(4):
                lo, hi = c * 1124, (c + 1) * 1124
                nc.sync.dma_start(x16s[ts][:, lo:hi], x16_d[ts][:, lo:hi])

        for ts in range(3):
            emit_xin(ts)

        # ---- prologue: kernel-predictor math (tiny, f32) ----
        W1_sb = const.tile([128, NSAMP, 3, 96], BF16)     # stage-1 weights
        bias_sb = const.tile([128, 4, NSAMP], F32)        # per-channel bias
        Sb_sb = const.tile([128, NSAMP], F32)             # S[n] on 128 parts
        d_sb = small.tile([8, NSAMP, 9], F32)
        dcol_sb = small.tile([128, NSAMP, 9], F32)
        ssum_sb = small.tile([128, 4, NSAMP], F32)
        pkwsum_sb = small.tile([128, 4], F32)
        pkbsum_sb = small.tile([1, 1], F32)
        S_sb = small.tile([1, NSAMP], F32)
        magic_sb = const.tile([128, 1], F32)
        nc.vector.memset(magic_sb[:], RSQRT_MAGIC)
        nc.vector.memset(W1_sb[:], 0.0)

        nc.vector.tensor_reduce(pkbsum_sb[:], pkb_sb[:], axis=AX.X, op=ALU.add)
        for kt in range(4):
            nc.vector.tensor_reduce(
                pkwsum_sb[:, kt:kt + 1], pkwT_sb[:, kt, :], axis=AX.X, op=ALU.add)

        for s in range(NSAMP):
            # d = leaky(conv2x2(style, dw_w) + dw_b):  16 accumulating matmuls
            psA0 = psumA.tile([128, 1024], F32, tag="psA")
            ps_d = psA0[0:8, 0:9]
            i = 0
            for ky in range(2):
                for kx in range(2):
                    for kt in range(4):
                        rhs = style_sb[:, s, kt, :].rearrange(
                            "p (y x) -> p y x", x=4)[:, ky:ky + 3, kx:kx + 3]
                        nc.tensor.matmul(
                            ps_d, dwT_sb[:, ky, kx, kt, :], rhs,
                            start=(i == 0), stop=(i == 15))
                        i += 1
            nc.scalar.activation(
                d_sb[:, s, :], ps_d, AF.Lrelu, bias=dwb_sb[:], alpha=0.01)

            # replicate d over channels: dcol[c,t] = d[c%8,t]
            psA1 = psumA.tile([128, 1024], F32, tag="psA")
            ps_dc = psA1[:, 0:9]
            nc.tensor.matmul(ps_dc, repl8_sb[:], d_sb[:, s, :])
            nc.vector.tensor_copy(dcol_sb[:, s, :], ps_dc)

            # stage-1 weights W1[kw][ch, kh*32+g] = d[ch%8, kh, kw]*(g==ch//8)
            for kh in range(3):
                for kw in range(3):
                    nc.vector.tensor_scalar(
                        W1_sb[:, s, kw, kh * 32: kh * 32 + 32], mask32_sb[:],
                        dcol_sb[:, s, 3 * kh + kw: 3 * kh + kw + 1], None,
                        ALU.mult)

            # style spatial sums (s_d * 16)
            for kt in range(4):
                nc.vector.tensor_reduce(
                    ssum_sb[:, kt, s:s + 1], style_sb[:, s, kt, :],
                    axis=AX.X, op=ALU.add)

        # bias[c] = s_d @ pb_w[c] + pb_b[c]   (both samples batched)
        for mt in range(4):
            psB0 = psumB.tile([128, 1024], F32, tag="psB")
            ps_b = psB0[:, 0:NSAMP]
            for kt in range(4):
                nc.tensor.matmul(
                    ps_b, pbT_sb[:, kt, mt * 128:(mt + 1) * 128],
                    ssum_sb[:, kt, :], start=(kt == 0), stop=(kt == 3))
            nc.vector.tensor_scalar(
                bias_sb[:, mt, :], ps_b, 1.0 / 16.0,
                pbb_sb[:, mt:mt + 1], ALU.mult, ALU.add)

        # S = s_d @ pkw_sum + sum(pk_b)
        psB1 = psumB.tile([128, 1024], F32, tag="psB")
        ps_S = psB1[0:1, 0:NSAMP]
        for kt in range(4):
            nc.tensor.matmul(
                ps_S, pkwsum_sb[:, kt:kt + 1], ssum_sb[:, kt, :],
                start=(kt == 0), stop=(kt == 3))
        nc.vector.tensor_scalar(
            S_sb[:], ps_S, 1.0 / 16.0, pkbsum_sb[:], ALU.mult, ALU.add)
        nc.gpsimd.partition_broadcast(Sb_sb[:], S_sb[:])

        # ---- per-tile state ----
        mv_all = small.tile([128, 8, 2], F32)        # (mean, var) per tile
        v_all = small.tile([128, 8], F32)            # var + eps
        y_all = small.tile([128, 8], F32)            # rsqrt iterate
        t_all = small.tile([128, 8], F32)
        scaleS_all = small.tile([128, 8], F32)       # S * rstd
        biasS_all = small.tile([128, 8], F32)        # bias * rstd

        def bn_stats_raw(out, in_):
            # bass's bn_stats wrapper mis-asserts the out shape for 3D
            # inputs; the HW op always writes 6 elements/partition.
            eng = nc.vector
            return eng.add_instruction(mybir.InstBNStats(
                name=eng.bass.get_next_instruction_name(),
                ins=[eng.lower_ap(in_)], outs=[eng.lower_ap(out)]))

        def emit_stats(ts):
            xr = x16s[ts][:, :EXT].rearrange("p (r w) -> p r w", w=PW)
            st = statp.tile([128, 8, 6], F32, tag="bn")
            for i in range(8):
                bn_stats_raw(st[:, i, :], xr[:, 1 + 8 * i: 9 + 8 * i, 2:66])
            nc.vector.bn_aggr(mv_all[:, ts, :], st[:])
            # xn = x - mean (bf16, dense)
            xn = xnp.tile([128, H * W], BF16, tag="xn")
            nc.vector.tensor_scalar(
                xn[:].rearrange("p (r w) -> p r w", w=64),
                xr[:, 1:65, 2:66], mv_all[:, ts, 0:1], None, ALU.subtract)
            return xn

        def emit_finalize(p):
            # pair-batched: tiles 2p, 2p+1
            sl = slice(2 * p, 2 * p + 2)
            s = (2 * p) // 4
            # v = var*4096/4095 + eps
            nc.vector.tensor_scalar(
                v_all[:, sl], mv_all[:, sl, 1], 4096.0 / 4095.0, 1e-5,
                ALU.mult, ALU.add)
            # Newton rsqrt: y0 from the bit trick, then 2 iterations
            nc.vector.tensor_scalar(
                t_all[:, sl].bitcast(I32), v_all[:, sl].bitcast(I32), 1,
                None, ALU.arith_shift_right)
            nc.vector.tensor_tensor(
                y_all[:, sl].bitcast(I32),
                magic_sb[:].bitcast(I32).to_broadcast([128, 2]),
                t_all[:, sl].bitcast(I32), ALU.subtract)
            y, t, v = y_all[:, sl], t_all[:, sl], v_all[:, sl]
            for _ in range(2):
                nc.vector.tensor_tensor(t, y, y, ALU.mult)
                nc.vector.tensor_tensor(t, t, v, ALU.mult)
                nc.vector.tensor_scalar(t, t, -0.5, 1.5, ALU.mult, ALU.add)
                nc.vector.tensor_tensor(y, y, t, ALU.mult)
            nc.vector.tensor_scalar(
                scaleS_all[:, sl], y, Sb_sb[:, s:s + 1], None, ALU.mult)
            kt0 = (2 * p) % 4
            nc.vector.tensor_tensor(
                biasS_all[:, sl], y, bias_sb[:, kt0:kt0 + 2, s], ALU.mult)

        def emit_stage1(ts):
            s = ts // 4
            x16 = x16s[ts]
            p2 = p2p.tile([96, DE], BF16, tag="p2")
            # row-aligned 7-row slots; 2 slots per 2-bank psum tile
            r0 = 0
            ti = 0
            while r0 < NR:
                psA = psumA.tile([128, 1024], F32, tag="psA")
                rows = []
                for sub in (0, 512):
                    nr = min(7, NR - r0 - sum(rows))
                    if nr <= 0:
                        break
                    rows.append(nr)
                for si, nr in enumerate(rows):
                    rr = r0 + (rows[0] if si else 0)
                    lo = rr * PW
                    cw = nr * PW
                    for kw in range(3):
                        nc.tensor.matmul(
                            psA[0:96, si * 512: si * 512 + cw],
                            W1_sb[:, s, kw, :],
                            x16[:, lo + kw: lo + kw + cw],
                            start=(kw == 0), stop=(kw == 2))
                nrt = sum(rows)
                # seam-dropping copy: [96, slot, row, 68 -> 64] -> dense
                if len(rows) == 2 and rows[0] == rows[1]:
                    src = psA[0:96, :].rearrange(
                        "p (u q) -> p u q", q=512)[:, :, :rows[0] * PW] \
                        .rearrange("p u (r w) -> p u r w", w=PW)[:, :, :, 1:65]
                    dst = p2[:, r0 * 64: (r0 + nrt) * 64]
                    cop = nc.scalar.copy if ts % 2 == 0 else nc.vector.tensor_copy
                    cop(dst, src)
                else:
                    off = 0
                    for si, nr in enumerate(rows):
                        src = psA[0:96, si * 512: si * 512 + nr * PW].rearrange(
                            "p (r w) -> p r w", w=PW)[:, :, 1:65]
                        dst = p2[:, (r0 + off) * 64: (r0 + off + nr) * 64]
                        cop = (nc.scalar.copy if ts % 2 == 0
                               else nc.vector.tensor_copy)
                        cop(dst, src)
                        off += nr
                r0 += nrt
                ti += 1
            # pre-apply kh row shifts (dense pitch 64)
            nc.gpsimd.dma_start(p2[16:32, 0:DE - 64], p2[32:48, 64:DE])
            nc.gpsimd.dma_start(p2[32:48, 0:DE - 128], p2[64:80, 128:DE])
            return p2

        def emit_stage2(ts, p2, xn):
            predn = prednp.tile([128, H * W], BF16, tag="pred")
            out_sb = outp.tile([128, H * W], BF16, tag="out")
            for half in range(2):
                for rt in (0, 16):
                    r0 = half * 32 + rt
                    psB = psumB.tile([128, 1024], F32, tag="psB")
                    for sub in (0, 512):
                        rr = r0 + (8 if sub else 0)
                        nc.tensor.matmul(
                            psB[:, sub:sub + 512], w2_sb[:],
                            p2[0:48, rr * 64: (rr + 8) * 64],
                            start=True, stop=True)
                    nc.scalar.activation(
                        predn[:, r0 * 64: (r0 + 16) * 64], psB[:, :],
                        AF.Lrelu, bias=biasS_all[:, ts:ts + 1],
                        scale=scaleS_all[:, ts:ts + 1], alpha=0.01)
                lo, hi = half * 2048, (half + 1) * 2048
                nc.vector.tensor_tensor(
                    out_sb[:, lo:hi], xn[:, lo:hi], predn[:, lo:hi], ALU.mult)
                nc.sync.dma_start(out_d[ts][:, lo:hi], out_sb[:, lo:hi])

        # ---- software-pipelined main loop: stage1(k+1) ahead of stage2(k) ----
        xn0 = emit_stats(0)
        p2_prev, xn_prev = emit_stage1(0), xn0
        for k in range(8):
            if k + 1 < 8:
                if k + 3 < 8:
                    emit_xin(k + 3)
                xn_next = emit_stats(k + 1)
                if (k + 1) % 2 == 1:
                    emit_finalize((k + 1) // 2)
                p2_next = emit_stage1(k + 1)
            emit_stage2(k, p2_prev, xn_prev)
            if k + 1 < 8:
                p2_prev, xn_prev = p2_next, xn_next

    nc.compile()
    return nc


def _host_prep(style_encoding, content_in, dw_w, dw_b, pk_w, pk_b, pb_w, pb_b):
    """Shard + lay out inputs for the 8 cores (layout only, no math)."""
    f32 = np.float32
    bf = ml_dtypes.bfloat16
    common = {
        "dwT": np.ascontiguousarray(
            dw_w.reshape(8, 4, 128, 2, 2).transpose(2, 3, 4, 1, 0), f32),
        "dwb": np.ascontiguousarray(dw_b.reshape(8, 1), f32),
        "pbT": np.ascontiguousarray(
            pb_w.T.reshape(4, 128, 512).transpose(1, 0, 2), f32),
        "pbb": np.ascontiguousarray(pb_b.reshape(4, 128).T, f32),
        "pkwT": np.ascontiguousarray(
            pk_w.T.reshape(4, 128, 8).transpose(1, 0, 2), f32),
        "pkb": np.ascontiguousarray(pk_b.reshape(1, 8), f32),
    }
    ii = np.arange(128)
    common["mask32"] = (np.arange(32)[None, :] == (ii[:, None] // 8)).astype(bf)
    w2 = np.zeros((48, 128), bf)
    for kh in range(3):
        w2[kh * 16 + ii // 8, ii] = 1
    common["w2"] = w2
    common["repl8"] = (np.arange(8)[:, None] == (ii[None, :] % 8)).astype(f32)

    # padded pitch-68 bf16 content, all cores at once
    xp = np.pad(content_in, ((0, 0), (0, 0), (1, 1), (1, 1)), mode="reflect")
    buf = np.zeros((16, CH, NR, PW), f32)
    buf[:, :, :, 1:67] = xp
    xb = buf.reshape(16, 4, 128, EXT).astype(bf)

    in_maps = []
    for i in range(N_CORES):
        x16 = np.zeros((NSAMP, 4, 128, XAL), bf)
        x16[:, :, :, :EXT] = xb[NSAMP * i: NSAMP * (i + 1)]
        se = style_encoding[NSAMP * i: NSAMP * (i + 1)]
        in_maps.append({
            "x16": np.ascontiguousarray(x16.reshape(8, 128, XAL)),
            "style": np.ascontiguousarray(
                se.reshape(NSAMP, 4, 128, 16).transpose(2, 0, 1, 3), f32),
            **common,
        })
    return in_maps


def kernel(style_encoding, content_in, dw_w, dw_b, pk_w, pk_b, pb_w, pb_b):
    global LAST_RESULTS
    import os
    if "nc" not in _CACHE:
        _CACHE["nc"] = _build()
    nc = _CACHE["nc"]
    in_maps = _host_prep(style_encoding, content_in, dw_w, dw_b,
                         pk_w, pk_b, pb_w, pb_b)
    res = run_bass_kernel_spmd(
        nc, in_maps, core_ids=list(range(N_CORES)),
        trace=bool(os.environ.get("ADACONV_TRACE")))
    LAST_RESULTS = res
    outs = []
    for i in range(N_CORES):
        o = np.asarray(res.results[i]["out"]).astype(np.float32)
        outs.append(o.reshape(NSAMP, 4, 128, 64, 64).reshape(NSAMP, CH, 64, 64))
    return np.concatenate(outs, axis=0)


# revision 17
# speedup vs baseline: 4.3341x; 1.1031x over previous
"""AdaConv kernel for 8 TRN2 NeuronCores — data-parallel over batch.

Two-stage stencil formulation. Math identical to the reference after
collapsing the per-sample grouped convs:
    D[n,g,h,w] = sum_{j,kh,kw} d[n,j,kh,kw] * xpad[n,8g+j,h+kh,w+kw]
    out = leaky(S[n]*D[n,c//8] + bias[n,c]) * (x - mean)/std

Per core (2 samples, 8 tiles of 128 channels; x stored bf16, pitch-68 rows):
  stage 1 (PE): 3 accumulating matmuls (kw taps via rhs column offsets),
      M = 96 = (kh,g) pairs 32-aligned -> P2[(kh,g), p] in PSUM.
  fold: PSUM->SBUF copy drops the seam cols (dense 64-pitch bf16), then two
      SBUF->SBUF DMAs shift the kh=1,2 slabs by kh*64 so taps align.
  stage 2 (PE): ONE K=48 matmul per 7-row slot -> D replicated over the 8
      channels of each group, directly in [128, px] dense layout.
  ScalarE evicts with fused leaky(S*rstd*D + bias*rstd) (valid: r>0).
  Stats on DVE: bn_stats/bn_aggr (bf16 x); rstd via Newton rsqrt on DVE
      (no ScalarE Sqrt -> no activation-table thrash).
  Final: DVE tensor_scalar xn = x - mean, tensor_tensor out = xn * predn.
"""

import numpy as np
import ml_dtypes
from contextlib import ExitStack

import concourse.bass as bass
import concourse.tile as tile
from concourse import bacc, mybir
from concourse.bass_utils import run_bass_kernel_spmd

F32 = mybir.dt.float32
I32 = mybir.dt.int32
BF16 = mybir.dt.bfloat16
AF = mybir.ActivationFunctionType
ALU = mybir.AluOpType
AX = mybir.AxisListType

N_CORES = 8
NSAMP = 2           # samples per core
CH = 512
H = W = 64
PW = 68             # row pitch (junk col 0, padded cols 1..66, junk col 67)
NR = 66             # padded rows
EXT = PW * NR       # 4488 flat extent
XAL = EXT + 8       # 4496 allocated (stencil over-read + even)
DE = 64 * NR        # 4224 dense P2 extent
RSQRT_MAGIC = np.int32(0x5F3759DF).view(np.float32).item()

LAST_RESULTS = None  # BassKernelResults of the most recent run (for test.py)
_CACHE = {}


def _build():
    nc = bacc.Bacc("TRN2", target_bir_lowering=False, debug=False)

    x16_d = nc.dram_tensor("x16", [8, 128, XAL], BF16, kind="ExternalInput")
    style_d = nc.dram_tensor("style", [128, NSAMP, 4, 16], F32, kind="ExternalInput")
    dwT_d = nc.dram_tensor("dwT", [128, 2, 2, 4, 8], F32, kind="ExternalInput")
    dwb_d = nc.dram_tensor("dwb", [8, 1], F32, kind="ExternalInput")
    pbT_d = nc.dram_tensor("pbT", [128, 4, 512], F32, kind="ExternalInput")
    pbb_d = nc.dram_tensor("pbb", [128, 4], F32, kind="ExternalInput")
    pkwT_d = nc.dram_tensor("pkwT", [128, 4, 8], F32, kind="ExternalInput")
    pkb_d = nc.dram_tensor("pkb", [1, 8], F32, kind="ExternalInput")
    mask16_d = nc.dram_tensor("mask16", [128, 16], BF16, kind="ExternalInput")
    w2_d = nc.dram_tensor("w2", [128, 128], BF16, kind="ExternalInput")
    repl8_d = nc.dram_tensor("repl8", [8, 128], F32, kind="ExternalInput")
    out_d = nc.dram_tensor("out", [8, 128, H * W], BF16, kind="ExternalOutput")

    with tile.TileContext(nc) as tc, ExitStack() as ctx:
        const = ctx.enter_context(tc.tile_pool(name="const", bufs=1))
        small = ctx.enter_context(tc.tile_pool(name="small", bufs=1))
        x16p = ctx.enter_context(tc.tile_pool(name="x16", bufs=8))
        p2p = ctx.enter_context(tc.tile_pool(name="p2", bufs=2))
        statp = ctx.enter_context(tc.tile_pool(name="stat", bufs=2))
        xnp = ctx.enter_context(tc.tile_pool(name="xn", bufs=4))
        prednp = ctx.enter_context(tc.tile_pool(name="pred", bufs=2))
        outp = ctx.enter_context(tc.tile_pool(name="outp", bufs=2))
        psumA = ctx.enter_context(
            tc.tile_pool(name="psumA", bufs=2, space="PSUM"))
        psumB = ctx.enter_context(
            tc.tile_pool(name="psumB", bufs=2, space="PSUM"))

        # ---- params (small, first on the sync queue) ----
        style_sb = const.tile([128, NSAMP, 4, 16], F32)
        nc.sync.dma_start(style_sb[:], style_d[:])
        dwT_sb = const.tile([128, 2, 2, 4, 8], F32)
        nc.sync.dma_start(dwT_sb[:], dwT_d[:])
        dwb_sb = const.tile([8, 1], F32)
        nc.sync.dma_start(dwb_sb[:], dwb_d[:])
        repl8_sb = const.tile([8, 128], F32)
        nc.sync.dma_start(repl8_sb[:], repl8_d[:])
        mask16_sb = const.tile([128, 16], BF16)
        nc.sync.dma_start(mask16_sb[:], mask16_d[:])
        w2_sb = const.tile([128, 128], BF16)
        nc.sync.dma_start(w2_sb[:], w2_d[:])
        pkb_sb = const.tile([1, 8], F32)
        nc.scalar.dma_start(pkb_sb[:], pkb_d[:])
        pbb_sb = const.tile([128, 4], F32)
        nc.scalar.dma_start(pbb_sb[:], pbb_d[:])
        pbT_sb = const.tile([128, 4, 512], F32)
        nc.scalar.dma_start(pbT_sb[:], pbT_d[:])
        pkwT_sb = const.tile([128, 4, 8], F32)
        nc.scalar.dma_start(pkwT_sb[:], pkwT_d[:])

        # content loads for the first tiles (prefetch window of 3)
        x16s = []
        for _ in range(8):
            x16 = x16p.tile([128, XAL], BF16, tag="x16")
            x16s.append(x16)

        def emit_xin(ts):
            for c in range(4):
                lo, hi = c * 1124, (c + 1) * 1124
                nc.sync.dma_start(x16s[ts][:, lo:hi], x16_d[ts][:, lo:hi])

        for ts in range(4):
            emit_xin(ts)

        # ---- prologue: kernel-predictor math (tiny, f32) ----
        W1_sb = const.tile([128, NSAMP, 3, 64], BF16)     # stage-1 weights
        bias_sb = const.tile([128, 4, NSAMP], F32)        # per-channel bias
        Sb_sb = const.tile([128, NSAMP], F32)             # S[n] on 128 parts
        d_sb = small.tile([8, NSAMP, 9], F32)
        dcol_sb = small.tile([128, NSAMP, 9], F32)
        ssum_sb = small.tile([128, 4, NSAMP], F32)
        pkwsum_sb = small.tile([128, 4], F32)
        pkbsum_sb = small.tile([1, 1], F32)
        S_sb = small.tile([1, NSAMP], F32)
        magic_sb = const.tile([128, 1], F32)
        nc.vector.memset(magic_sb[:], RSQRT_MAGIC)
        nc.vector.memset(W1_sb[:], 0.0)

        nc.vector.tensor_reduce(pkbsum_sb[:], pkb_sb[:], axis=AX.X, op=ALU.add)
        for kt in range(4):
            nc.vector.tensor_reduce(
                pkwsum_sb[:, kt:kt + 1], pkwT_sb[:, kt, :], axis=AX.X, op=ALU.add)

        for s in range(NSAMP):
            # d = leaky(conv2x2(style, dw_w) + dw_b):  16 accumulating matmuls
            psA0 = psumA.tile([128, 1024], F32, tag="psA")
            ps_d = psA0[0:8, 0:9]
            i = 0
            for ky in range(2):
                for kx in range(2):
                    for kt in range(4):
                        rhs = style_sb[:, s, kt, :].rearrange(
                            "p (y x) -> p y x", x=4)[:, ky:ky + 3, kx:kx + 3]
                        nc.tensor.matmul(
                            ps_d, dwT_sb[:, ky, kx, kt, :], rhs,
                            start=(i == 0), stop=(i == 15))
                        i += 1
            nc.scalar.activation(
                d_sb[:, s, :], ps_d, AF.Lrelu, bias=dwb_sb[:], alpha=0.01)

            # replicate d over channels: dcol[c,t] = d[c%8,t]
            psA1 = psumA.tile([128, 1024], F32, tag="psA")
            ps_dc = psA1[:, 0:9]
            nc.tensor.matmul(ps_dc, repl8_sb[:], d_sb[:, s, :])
            nc.vector.tensor_copy(dcol_sb[:, s, :], ps_dc)

            # stage-1 weights W1[kw][ch, kh*16+g] = d[ch%8, kh, kw]*(g==ch//8)
            for kh in range(3):
                for kw in range(3):
                    nc.vector.tensor_scalar(
                        W1_sb[:, s, kw, kh * 16: kh * 16 + 16], mask16_sb[:],
                        dcol_sb[:, s, 3 * kh + kw: 3 * kh + kw + 1], None,
                        ALU.mult)

            # style spatial sums (s_d * 16)
            for kt in range(4):
                nc.vector.tensor_reduce(
                    ssum_sb[:, kt, s:s + 1], style_sb[:, s, kt, :],
                    axis=AX.X, op=ALU.add)

        # bias[c] = s_d @ pb_w[c] + pb_b[c]   (both samples batched)
        for mt in range(4):
            psB0 = psumB.tile([128, 1024], F32, tag="psB")
            ps_b = psB0[:, 0:NSAMP]
            for kt in range(4):
                nc.tensor.matmul(
                    ps_b, pbT_sb[:, kt, mt * 128:(mt + 1) * 128],
                    ssum_sb[:, kt, :], start=(kt == 0), stop=(kt == 3))
            nc.vector.tensor_scalar(
                bias_sb[:, mt, :], ps_b, 1.0 / 16.0,
                pbb_sb[:, mt:mt + 1], ALU.mult, ALU.add)

        # S = s_d @ pkw_sum + sum(pk_b)
        psB1 = psumB.tile([128, 1024], F32, tag="psB")
        ps_S = psB1[0:1, 0:NSAMP]
        for kt in range(4):
            nc.tensor.matmul(
                ps_S, pkwsum_sb[:, kt:kt + 1], ssum_sb[:, kt, :],
                start=(kt == 0), stop=(kt == 3))
        nc.vector.tensor_scalar(
            S_sb[:], ps_S, 1.0 / 16.0, pkbsum_sb[:], ALU.mult, ALU.add)
        nc.gpsimd.partition_broadcast(Sb_sb[:], S_sb[:])

        # ---- per-tile state ----
        mv_all = small.tile([128, 8, 2], F32)        # (mean, var) per tile
        negmu_all = small.tile([128, 8], F32)
        v_all = small.tile([128, 8], F32)            # var + eps
        y_all = small.tile([128, 8], F32)            # rsqrt iterate
        t_all = small.tile([128, 8], F32)
        scaleS_all = small.tile([128, 8], F32)       # S * rstd
        biasS_all = small.tile([128, 8], F32)        # bias * rstd

        def bn_stats_raw(out, in_):
            # bass's bn_stats wrapper mis-asserts the out shape for 3D
            # inputs; the HW op always writes 6 elements/partition.
            eng = nc.vector
            return eng.add_instruction(mybir.InstBNStats(
                name=eng.bass.get_next_instruction_name(),
                ins=[eng.lower_ap(in_)], outs=[eng.lower_ap(out)]))

        def emit_stats(ts):
            xr = x16s[ts][:, :EXT].rearrange("p (r w) -> p r w", w=PW)
            st = statp.tile([128, 8, 6], F32, tag="bn")
            for i in range(8):
                bn_stats_raw(st[:, i, :], xr[:, 1 + 8 * i: 9 + 8 * i, 2:66])
            nc.vector.bn_aggr(mv_all[:, ts, :], st[:])

        def emit_finalize(p):
            # pair-batched: tiles 2p, 2p+1
            sl = slice(2 * p, 2 * p + 2)
            s = (2 * p) // 4
            nc.vector.tensor_scalar(
                negmu_all[:, sl], mv_all[:, sl, 0], -1.0, None, ALU.mult)
            # v = var*4096/4095 + eps
            nc.vector.tensor_scalar(
                v_all[:, sl], mv_all[:, sl, 1], 4096.0 / 4095.0, 1e-5,
                ALU.mult, ALU.add)
            # Newton rsqrt: y0 from the bit trick, then 2 iterations
            nc.vector.tensor_scalar(
                t_all[:, sl].bitcast(I32), v_all[:, sl].bitcast(I32), 1,
                None, ALU.arith_shift_right)
            nc.vector.tensor_tensor(
                y_all[:, sl].bitcast(I32),
                magic_sb[:].bitcast(I32).to_broadcast([128, 2]),
                t_all[:, sl].bitcast(I32), ALU.subtract)
            y, t, v = y_all[:, sl], t_all[:, sl], v_all[:, sl]
            for _ in range(2):
                nc.vector.tensor_tensor(t, y, y, ALU.mult)
                nc.vector.tensor_tensor(t, t, v, ALU.mult)
                nc.vector.tensor_scalar(t, t, -0.5, 1.5, ALU.mult, ALU.add)
                nc.vector.tensor_tensor(y, y, t, ALU.mult)
            nc.vector.tensor_scalar(
                scaleS_all[:, sl], y, Sb_sb[:, s:s + 1], None, ALU.mult)
            kt0 = (2 * p) % 4
            nc.vector.tensor_tensor(
                biasS_all[:, sl], y, bias_sb[:, kt0:kt0 + 2, s], ALU.mult)

        def emit_xn(ts):
            # xn = x - mean on ScalarE (Identity with bias = -mean)
            xr = x16s[ts][:, :EXT].rearrange("p (r w) -> p r w", w=PW)
            xn = xnp.tile([128, H * W], BF16, tag="xn")
            nc.scalar.activation(
                xn[:].rearrange("p (r w) -> p r w", w=64),
                xr[:, 1:65, 2:66], AF.Identity,
                bias=negmu_all[:, ts:ts + 1])
            return xn

        def emit_stage1_pair(p):
            # tiles a = 2p (psum cols 0:48), b = 2p+1 (psum cols 64:112)
            a, b = 2 * p, 2 * p + 1
            s = a // 4
            xa, xb = x16s[a], x16s[b]
            p2 = p2p.tile([128, DE], BF16, tag="p2")
            r0 = 0
            while r0 < NR:
                nr0 = min(7, NR - r0)
                nr1 = min(7, NR - r0 - nr0)
                rows = [nr0] + ([nr1] if nr1 > 0 else [])
                psA = psumA.tile([128, 1024], F32, tag="psA")
                for kw in range(3):
                    for si, nr in enumerate(rows):
                        rr = r0 + (rows[0] if si else 0)
                        lo = rr * PW + kw
                        cw = nr * PW
                        nc.tensor.matmul(
                            psA[0:64, si * 512: si * 512 + cw],
                            W1_sb[:, s, kw, :], xa[:, lo: lo + cw],
                            start=(kw == 0), stop=(kw == 2),
                            skip_group_check=True)
                        nc.tensor.matmul(
                            psA[64:128, si * 512: si * 512 + cw],
                            W1_sb[:, s, kw, :], xb[:, lo: lo + cw],
                            start=(kw == 0), stop=(kw == 2),
                            skip_group_check=True)
                nrt = sum(rows)
                # one seam-dropping copy evacuates BOTH tiles
                if len(rows) == 2 and rows[0] == rows[1]:
                    srcv = psA[0:112, :].rearrange(
                        "p (u q) -> p u q", q=512)[:, :, :rows[0] * PW]                         .rearrange("p u (r w) -> p u r w", w=PW)[:, :, :, 1:65]
                    nc.scalar.copy(p2[0:112, r0 * 64: (r0 + nrt) * 64], srcv)
                else:
                    off = 0
                    for si, nr in enumerate(rows):
                        srcv = psA[0:112, si * 512: si * 512 + nr * PW]                             .rearrange("p (r w) -> p r w", w=PW)[:, :, 1:65]
                        nc.scalar.copy(
                            p2[0:112, (r0 + off) * 64: (r0 + off + nr) * 64],
                            srcv)
                        off += nr
                r0 += nrt
            # slab shifts into the gap rows (kh1 -> rows 48:64, kh2 -> 16:32)
            for base in (0, 64):
                nc.gpsimd.dma_start(
                    p2[base + 48: base + 64, 0: DE - 64],
                    p2[base + 16: base + 32, 64: DE])
                nc.gpsimd.dma_start(
                    p2[base + 16: base + 32, 0: DE - 128],
                    p2[base + 32: base + 48, 128: DE])
            return p2

        def emit_stage2(ts, p2, xn):
            half64 = 64 * (ts % 2)      # a -> rows 0:64, b -> rows 64:128
            predn = prednp.tile([128, H * W], BF16, tag="pred")
            out_sb = outp.tile([128, H * W], BF16, tag="out")
            for half in range(2):
                for rt in (0, 16):
                    r0 = half * 32 + rt
                    psB = psumB.tile([128, 1024], F32, tag="psB")
                    for sub in (0, 512):
                        rr = r0 + (8 if sub else 0)
                        nc.tensor.matmul(
                            psB[:, sub:sub + 512],
                            w2_sb[half64: half64 + 64, :],
                            p2[half64: half64 + 64, rr * 64: (rr + 8) * 64],
                            start=True, stop=True)
                    nc.scalar.activation(
                        predn[:, r0 * 64: (r0 + 16) * 64], psB[:, :],
                        AF.Lrelu, bias=biasS_all[:, ts:ts + 1],
                        scale=scaleS_all[:, ts:ts + 1], alpha=0.01)
                lo, hi = half * 2048, (half + 1) * 2048
                nc.vector.tensor_tensor(
                    out_sb[:, lo:hi], xn[:, lo:hi], predn[:, lo:hi], ALU.mult)
                nc.sync.dma_start(out_d[ts][:, lo:hi], out_sb[:, lo:hi])

        # ---- software-pipelined main loop over tile pairs ----
        emit_stats(0)
        emit_stats(1)
        emit_finalize(0)
        xn_prev = [emit_xn(0), emit_xn(1)]
        p2_prev = emit_stage1_pair(0)
        for p in range(4):
            if p + 1 < 4:
                for ts in (2 * p + 4, 2 * p + 5):
                    if ts < 8:
                        emit_xin(ts)
                emit_stats(2 * p + 2)
                emit_stats(2 * p + 3)
                emit_finalize(p + 1)
                xn_next = [emit_xn(2 * p + 2), emit_xn(2 * p + 3)]
                p2_next = emit_stage1_pair(p + 1)
            emit_stage2(2 * p, p2_prev, xn_prev[0])
            emit_stage2(2 * p + 1, p2_prev, xn_prev[1])
            if p + 1 < 4:
                p2_prev, xn_prev = p2_next, xn_next

    nc.compile()
    return nc


def _host_prep(style_encoding, content_in, dw_w, dw_b, pk_w, pk_b, pb_w, pb_b):
    """Shard + lay out inputs for the 8 cores (layout only, no math)."""
    f32 = np.float32
    bf = ml_dtypes.bfloat16
    common = {
        "dwT": np.ascontiguousarray(
            dw_w.reshape(8, 4, 128, 2, 2).transpose(2, 3, 4, 1, 0), f32),
        "dwb": np.ascontiguousarray(dw_b.reshape(8, 1), f32),
        "pbT": np.ascontiguousarray(
            pb_w.T.reshape(4, 128, 512).transpose(1, 0, 2), f32),
        "pbb": np.ascontiguousarray(pb_b.reshape(4, 128).T, f32),
        "pkwT": np.ascontiguousarray(
            pk_w.T.reshape(4, 128, 8).transpose(1, 0, 2), f32),
        "pkb": np.ascontiguousarray(pk_b.reshape(1, 8), f32),
    }
    ii = np.arange(128)
    common["mask16"] = (np.arange(16)[None, :] == (ii[:, None] // 8)).astype(bf)
    # post-shift row order: kh0 at rows 0:16, kh2 at 16:32, kh1 at 48:64
    w2 = np.zeros((128, 128), bf)
    for base in (0, 64):
        w2[base + 0 + ii // 8, ii] = 1     # kh = 0
        w2[base + 16 + ii // 8, ii] = 1    # kh = 2 (shifted into 16:32)
        w2[base + 48 + ii // 8, ii] = 1    # kh = 1 (shifted into 48:64)
    common["w2"] = w2
    common["repl8"] = (np.arange(8)[:, None] == (ii[None, :] % 8)).astype(f32)

    # padded pitch-68 bf16 content, all cores at once
    xp = np.pad(content_in, ((0, 0), (0, 0), (1, 1), (1, 1)), mode="reflect")
    buf = np.zeros((16, CH, NR, PW), f32)
    buf[:, :, :, 1:67] = xp
    xb = buf.reshape(16, 4, 128, EXT).astype(bf)

    in_maps = []
    for i in range(N_CORES):
        x16 = np.zeros((NSAMP, 4, 128, XAL), bf)
        x16[:, :, :, :EXT] = xb[NSAMP * i: NSAMP * (i + 1)]
        se = style_encoding[NSAMP * i: NSAMP * (i + 1)]
        in_maps.append({
            "x16": np.ascontiguousarray(x16.reshape(8, 128, XAL)),
            "style": np.ascontiguousarray(
                se.reshape(NSAMP, 4, 128, 16).transpose(2, 0, 1, 3), f32),
            **common,
        })
    return in_maps


def kernel(style_encoding, content_in, dw_w, dw_b, pk_w, pk_b, pb_w, pb_b):
    global LAST_RESULTS
    import os
    if "nc" not in _CACHE:
        _CACHE["nc"] = _build()
    nc = _CACHE["nc"]
    in_maps = _host_prep(style_encoding, content_in, dw_w, dw_b,
                         pk_w, pk_b, pb_w, pb_b)
    res = run_bass_kernel_spmd(
        nc, in_maps, core_ids=list(range(N_CORES)),
        trace=bool(os.environ.get("ADACONV_TRACE")))
    LAST_RESULTS = res
    outs = []
    for i in range(N_CORES):
        o = np.asarray(res.results[i]["out"]).astype(np.float32)
        outs.append(o.reshape(NSAMP, 4, 128, 64, 64).reshape(NSAMP, CH, 64, 64))
    return np.concatenate(outs, axis=0)


# revision 18
# speedup vs baseline: 4.6025x; 1.0619x over previous
"""AdaConv kernel for 8 TRN2 NeuronCores — data-parallel over batch.

Two-stage stencil formulation. Math identical to the reference after
collapsing the per-sample grouped convs:
    D[n,g,h,w] = sum_{j,kh,kw} d[n,j,kh,kw] * xpad[n,8g+j,h+kh,w+kw]
    out = leaky(S[n]*D[n,c//8] + bias[n,c]) * (x - mean)/std

Per core (2 samples, 8 tiles of 128 channels; x stored bf16, pitch-68 rows):
  stage 1 (PE): 3 accumulating matmuls (kw taps via rhs column offsets),
      M = 96 = (kh,g) pairs 32-aligned -> P2[(kh,g), p] in PSUM.
  fold: PSUM->SBUF copy drops the seam cols (dense 64-pitch bf16), then two
      SBUF->SBUF DMAs shift the kh=1,2 slabs by kh*64 so taps align.
  stage 2 (PE): ONE K=48 matmul per 7-row slot -> D replicated over the 8
      channels of each group, directly in [128, px] dense layout.
  ScalarE evicts with fused leaky(S*rstd*D + bias*rstd) (valid: r>0).
  Stats on DVE: bn_stats/bn_aggr (bf16 x); rstd via Newton rsqrt on DVE
      (no ScalarE Sqrt -> no activation-table thrash).
  Final: DVE tensor_scalar xn = x - mean, tensor_tensor out = xn * predn.
"""

import numpy as np
import ml_dtypes
from contextlib import ExitStack

import concourse.bass as bass
import concourse.tile as tile
from concourse import bacc, mybir
from concourse.bass_utils import run_bass_kernel_spmd

F32 = mybir.dt.float32
I32 = mybir.dt.int32
BF16 = mybir.dt.bfloat16
AF = mybir.ActivationFunctionType
ALU = mybir.AluOpType
AX = mybir.AxisListType

N_CORES = 8
NSAMP = 2           # samples per core
CH = 512
H = W = 64
PW = 68             # row pitch (junk col 0, padded cols 1..66, junk col 67)
NR = 66             # padded rows
EXT = PW * NR       # 4488 flat extent
XAL = EXT + 8       # 4496 allocated (stencil over-read + even)
DE = 64 * NR        # 4224 dense P2 extent
RSQRT_MAGIC = np.int32(0x5F3759DF).view(np.float32).item()

LAST_RESULTS = None  # BassKernelResults of the most recent run (for test.py)
_CACHE = {}


def _build():
    nc = bacc.Bacc("TRN2", target_bir_lowering=False, debug=False)

    x16_d = nc.dram_tensor("x16", [8, 128, XAL], BF16, kind="ExternalInput")
    style_d = nc.dram_tensor("style", [128, NSAMP, 4, 16], F32, kind="ExternalInput")
    dwT_d = nc.dram_tensor("dwT", [128, 2, 2, 4, 8], F32, kind="ExternalInput")
    dwb_d = nc.dram_tensor("dwb", [8, 1], F32, kind="ExternalInput")
    pbT_d = nc.dram_tensor("pbT", [128, 4, 512], F32, kind="ExternalInput")
    pbb_d = nc.dram_tensor("pbb", [128, 4], F32, kind="ExternalInput")
    pkwT_d = nc.dram_tensor("pkwT", [128, 4, 8], F32, kind="ExternalInput")
    pkb_d = nc.dram_tensor("pkb", [1, 8], F32, kind="ExternalInput")
    mask16_d = nc.dram_tensor("mask16", [128, 16], BF16, kind="ExternalInput")
    w2_d = nc.dram_tensor("w2", [128, 128], BF16, kind="ExternalInput")
    repl8_d = nc.dram_tensor("repl8", [8, 128], F32, kind="ExternalInput")
    out_d = nc.dram_tensor("out", [8, 128, H * W], BF16, kind="ExternalOutput")

    with tile.TileContext(nc) as tc, ExitStack() as ctx:
        const = ctx.enter_context(tc.tile_pool(name="const", bufs=1))
        small = ctx.enter_context(tc.tile_pool(name="small", bufs=1))
        x16p = ctx.enter_context(tc.tile_pool(name="x16", bufs=8))
        p2p = ctx.enter_context(tc.tile_pool(name="p2", bufs=2))
        statp = ctx.enter_context(tc.tile_pool(name="stat", bufs=2))
        xnp = ctx.enter_context(tc.tile_pool(name="xn", bufs=2))
        prednp = ctx.enter_context(tc.tile_pool(name="pred", bufs=2))
        outp = ctx.enter_context(tc.tile_pool(name="outp", bufs=2))
        psumA = ctx.enter_context(
            tc.tile_pool(name="psumA", bufs=2, space="PSUM"))
        psumB = ctx.enter_context(
            tc.tile_pool(name="psumB", bufs=2, space="PSUM"))

        # ---- params (small, first on the sync queue) ----
        style_sb = const.tile([128, NSAMP, 4, 16], F32)
        nc.sync.dma_start(style_sb[:], style_d[:])
        dwT_sb = const.tile([128, 2, 2, 4, 8], F32)
        nc.sync.dma_start(dwT_sb[:], dwT_d[:])
        dwb_sb = const.tile([8, 1], F32)
        nc.sync.dma_start(dwb_sb[:], dwb_d[:])
        repl8_sb = const.tile([8, 128], F32)
        nc.sync.dma_start(repl8_sb[:], repl8_d[:])
        mask16_sb = const.tile([128, 16], BF16)
        nc.sync.dma_start(mask16_sb[:], mask16_d[:])
        w2_sb = const.tile([128, 128], BF16)
        nc.sync.dma_start(w2_sb[:], w2_d[:])
        pkb_sb = const.tile([1, 8], F32)
        nc.scalar.dma_start(pkb_sb[:], pkb_d[:])
        pbb_sb = const.tile([128, 4], F32)
        nc.scalar.dma_start(pbb_sb[:], pbb_d[:])
        pbT_sb = const.tile([128, 4, 512], F32)
        nc.scalar.dma_start(pbT_sb[:], pbT_d[:])
        pkwT_sb = const.tile([128, 4, 8], F32)
        nc.scalar.dma_start(pkwT_sb[:], pkwT_d[:])

        # content loads for the first tiles (prefetch window of 3)
        x16s = []
        for _ in range(8):
            x16 = x16p.tile([128, XAL], BF16, tag="x16")
            x16s.append(x16)

        def emit_xin(ts):
            for c in range(4):
                lo, hi = c * 1124, (c + 1) * 1124
                nc.sync.dma_start(x16s[ts][:, lo:hi], x16_d[ts][:, lo:hi])

        for ts in range(4):
            emit_xin(ts)

        # ---- prologue: kernel-predictor math (tiny, f32) ----
        W1_sb = const.tile([128, NSAMP, 3, 64], BF16)     # stage-1 weights
        bias_sb = const.tile([128, 4, NSAMP], F32)        # per-channel bias
        Sb_sb = const.tile([128, NSAMP], F32)             # S[n] on 128 parts
        d_sb = small.tile([8, NSAMP, 9], F32)
        dcol_sb = small.tile([128, NSAMP, 9], F32)
        ssum_sb = small.tile([128, 4, NSAMP], F32)
        pkwsum_sb = small.tile([128, 4], F32)
        pkbsum_sb = small.tile([1, 1], F32)
        S_sb = small.tile([1, NSAMP], F32)
        magic_sb = const.tile([128, 1], F32)
        nc.vector.memset(magic_sb[:], RSQRT_MAGIC)
        nc.vector.memset(W1_sb[:], 0.0)

        nc.vector.tensor_reduce(pkbsum_sb[:], pkb_sb[:], axis=AX.X, op=ALU.add)
        for kt in range(4):
            nc.vector.tensor_reduce(
                pkwsum_sb[:, kt:kt + 1], pkwT_sb[:, kt, :], axis=AX.X, op=ALU.add)

        for s in range(NSAMP):
            # d = leaky(conv2x2(style, dw_w) + dw_b):  16 accumulating matmuls
            psA0 = psumA.tile([128, 1024], F32, tag="psA")
            ps_d = psA0[0:8, 0:9]
            i = 0
            for ky in range(2):
                for kx in range(2):
                    for kt in range(4):
                        rhs = style_sb[:, s, kt, :].rearrange(
                            "p (y x) -> p y x", x=4)[:, ky:ky + 3, kx:kx + 3]
                        nc.tensor.matmul(
                            ps_d, dwT_sb[:, ky, kx, kt, :], rhs,
                            start=(i == 0), stop=(i == 15))
                        i += 1
            nc.scalar.activation(
                d_sb[:, s, :], ps_d, AF.Lrelu, bias=dwb_sb[:], alpha=0.01)

            # replicate d over channels: dcol[c,t] = d[c%8,t]
            psA1 = psumA.tile([128, 1024], F32, tag="psA")
            ps_dc = psA1[:, 0:9]
            nc.tensor.matmul(ps_dc, repl8_sb[:], d_sb[:, s, :])
            nc.vector.tensor_copy(dcol_sb[:, s, :], ps_dc)

            # stage-1 weights W1[kw][ch, kh*16+g] = d[ch%8, kh, kw]*(g==ch//8)
            for kh in range(3):
                for kw in range(3):
                    nc.vector.tensor_scalar(
                        W1_sb[:, s, kw, kh * 16: kh * 16 + 16], mask16_sb[:],
                        dcol_sb[:, s, 3 * kh + kw: 3 * kh + kw + 1], None,
                        ALU.mult)

            # style spatial sums (s_d * 16)
            for kt in range(4):
                nc.vector.tensor_reduce(
                    ssum_sb[:, kt, s:s + 1], style_sb[:, s, kt, :],
                    axis=AX.X, op=ALU.add)

        # bias[c] = s_d @ pb_w[c] + pb_b[c]   (both samples batched)
        for mt in range(4):
            psB0 = psumB.tile([128, 1024], F32, tag="psB")
            ps_b = psB0[:, 0:NSAMP]
            for kt in range(4):
                nc.tensor.matmul(
                    ps_b, pbT_sb[:, kt, mt * 128:(mt + 1) * 128],
                    ssum_sb[:, kt, :], start=(kt == 0), stop=(kt == 3))
            nc.vector.tensor_scalar(
                bias_sb[:, mt, :], ps_b, 1.0 / 16.0,
                pbb_sb[:, mt:mt + 1], ALU.mult, ALU.add)

        # S = s_d @ pkw_sum + sum(pk_b)
        psB1 = psumB.tile([128, 1024], F32, tag="psB")
        ps_S = psB1[0:1, 0:NSAMP]
        for kt in range(4):
            nc.tensor.matmul(
                ps_S, pkwsum_sb[:, kt:kt + 1], ssum_sb[:, kt, :],
                start=(kt == 0), stop=(kt == 3))
        nc.vector.tensor_scalar(
            S_sb[:], ps_S, 1.0 / 16.0, pkbsum_sb[:], ALU.mult, ALU.add)
        nc.gpsimd.partition_broadcast(Sb_sb[:], S_sb[:])

        # ---- per-tile state ----
        mv_all = small.tile([128, 8, 2], F32)        # (mean, var) per tile
        v_all = small.tile([128, 8], F32)            # var + eps
        y_all = small.tile([128, 8], F32)            # rsqrt iterate
        t_all = small.tile([128, 8], F32)
        scaleS_all = small.tile([128, 8], F32)       # S * rstd
        biasS_all = small.tile([128, 8], F32)        # bias * rstd

        def bn_stats_raw(out, in_):
            # bass's bn_stats wrapper mis-asserts the out shape for 3D
            # inputs; the HW op always writes 6 elements/partition.
            eng = nc.vector
            return eng.add_instruction(mybir.InstBNStats(
                name=eng.bass.get_next_instruction_name(),
                ins=[eng.lower_ap(in_)], outs=[eng.lower_ap(out)]))

        def emit_stats(ts):
            xr = x16s[ts][:, :EXT].rearrange("p (r w) -> p r w", w=PW)
            st = statp.tile([128, 8, 6], F32, tag="bn")
            for i in range(8):
                bn_stats_raw(st[:, i, :], xr[:, 1 + 8 * i: 9 + 8 * i, 2:66])
            nc.vector.bn_aggr(mv_all[:, ts, :], st[:])

        def emit_finalize(p):
            # pair-batched: tiles 2p, 2p+1
            sl = slice(2 * p, 2 * p + 2)
            s = (2 * p) // 4
            # v = var*4096/4095 + eps
            nc.vector.tensor_scalar(
                v_all[:, sl], mv_all[:, sl, 1], 4096.0 / 4095.0, 1e-5,
                ALU.mult, ALU.add)
            # Newton rsqrt: y0 from the bit trick, then 2 iterations
            nc.vector.tensor_scalar(
                t_all[:, sl].bitcast(I32), v_all[:, sl].bitcast(I32), 1,
                None, ALU.arith_shift_right)
            nc.vector.tensor_tensor(
                y_all[:, sl].bitcast(I32),
                magic_sb[:].bitcast(I32).to_broadcast([128, 2]),
                t_all[:, sl].bitcast(I32), ALU.subtract)
            y, t, v = y_all[:, sl], t_all[:, sl], v_all[:, sl]
            for _ in range(2):
                nc.vector.tensor_tensor(t, y, y, ALU.mult)
                nc.vector.tensor_tensor(t, t, v, ALU.mult)
                nc.vector.tensor_scalar(t, t, -0.5, 1.5, ALU.mult, ALU.add)
                nc.vector.tensor_tensor(y, y, t, ALU.mult)
            nc.vector.tensor_scalar(
                scaleS_all[:, sl], y, Sb_sb[:, s:s + 1], None, ALU.mult)
            kt0 = (2 * p) % 4
            nc.vector.tensor_tensor(
                biasS_all[:, sl], y, bias_sb[:, kt0:kt0 + 2, s], ALU.mult)

        def emit_stage1_pair(p):
            # tiles a = 2p (psum cols 0:48), b = 2p+1 (psum cols 64:112)
            a, b = 2 * p, 2 * p + 1
            s = a // 4
            xa, xb = x16s[a], x16s[b]
            p2 = p2p.tile([128, DE], BF16, tag="p2")
            r0 = 0
            while r0 < NR:
                nr0 = min(7, NR - r0)
                nr1 = min(7, NR - r0 - nr0)
                rows = [nr0] + ([nr1] if nr1 > 0 else [])
                psA = psumA.tile([128, 1024], F32, tag="psA")
                for kw in range(3):
                    for si, nr in enumerate(rows):
                        rr = r0 + (rows[0] if si else 0)
                        lo = rr * PW + kw
                        cw = nr * PW
                        nc.tensor.matmul(
                            psA[0:64, si * 512: si * 512 + cw],
                            W1_sb[:, s, kw, :], xa[:, lo: lo + cw],
                            start=(kw == 0), stop=(kw == 2),
                            skip_group_check=True)
                        nc.tensor.matmul(
                            psA[64:128, si * 512: si * 512 + cw],
                            W1_sb[:, s, kw, :], xb[:, lo: lo + cw],
                            start=(kw == 0), stop=(kw == 2),
                            skip_group_check=True)
                nrt = sum(rows)
                # one seam-dropping copy evacuates BOTH tiles
                if len(rows) == 2 and rows[0] == rows[1]:
                    srcv = psA[0:112, :].rearrange(
                        "p (u q) -> p u q", q=512)[:, :, :rows[0] * PW]                         .rearrange("p u (r w) -> p u r w", w=PW)[:, :, :, 1:65]
                    nc.scalar.copy(p2[0:112, r0 * 64: (r0 + nrt) * 64], srcv)
                else:
                    off = 0
                    for si, nr in enumerate(rows):
                        srcv = psA[0:112, si * 512: si * 512 + nr * PW]                             .rearrange("p (r w) -> p r w", w=PW)[:, :, 1:65]
                        nc.scalar.copy(
                            p2[0:112, (r0 + off) * 64: (r0 + off + nr) * 64],
                            srcv)
                        off += nr
                r0 += nrt
            # slab shifts into the gap rows (kh1 -> rows 48:64, kh2 -> 16:32)
            for base in (0, 64):
                nc.gpsimd.dma_start(
                    p2[base + 48: base + 64, 0: DE - 64],
                    p2[base + 16: base + 32, 64: DE])
                nc.gpsimd.dma_start(
                    p2[base + 16: base + 32, 0: DE - 128],
                    p2[base + 32: base + 48, 128: DE])
            return p2

        def emit_stage2(ts, p2):
            half64 = 64 * (ts % 2)      # a -> rows 0:64, b -> rows 64:128
            xr = x16s[ts][:, :EXT].rearrange("p (r w) -> p r w", w=PW)
            predn = prednp.tile([128, H * W], BF16, tag="pred")
            xn = xnp.tile([128, H * W], BF16, tag="xn")
            out_sb = outp.tile([128, H * W], BF16, tag="out")
            for half in range(2):
                for rt in (0, 16):
                    r0 = half * 32 + rt
                    psB = psumB.tile([128, 1024], F32, tag="psB")
                    for sub in (0, 512):
                        rr = r0 + (8 if sub else 0)
                        nc.tensor.matmul(
                            psB[:, sub:sub + 512],
                            w2_sb[half64: half64 + 64, :],
                            p2[half64: half64 + 64, rr * 64: (rr + 8) * 64],
                            start=True, stop=True)
                    nc.scalar.activation(
                        predn[:, r0 * 64: (r0 + 16) * 64], psB[:, :],
                        AF.Lrelu, bias=biasS_all[:, ts:ts + 1],
                        scale=scaleS_all[:, ts:ts + 1], alpha=0.01)
                lo, hi = half * 2048, (half + 1) * 2048
                r0 = half * 32
                nc.vector.tensor_scalar(
                    xn[:, lo:hi].rearrange("p (r w) -> p r w", w=64),
                    xr[:, 1 + r0: 33 + r0, 2:66], mv_all[:, ts, 0:1], None,
                    ALU.subtract)
                nc.vector.tensor_tensor(
                    out_sb[:, lo:hi], xn[:, lo:hi], predn[:, lo:hi], ALU.mult)
                nc.sync.dma_start(out_d[ts][:, lo:hi], out_sb[:, lo:hi])

        # ---- software-pipelined main loop over tile pairs ----
        emit_stats(0)
        emit_stats(1)
        emit_finalize(0)
        p2_prev = emit_stage1_pair(0)
        for p in range(4):
            if p + 1 < 4:
                for ts in (2 * p + 4, 2 * p + 5):
                    if ts < 8:
                        emit_xin(ts)
                emit_stats(2 * p + 2)
                emit_stats(2 * p + 3)
                emit_finalize(p + 1)
                p2_next = emit_stage1_pair(p + 1)
            emit_stage2(2 * p, p2_prev)
            emit_stage2(2 * p + 1, p2_prev)
            if p + 1 < 4:
                p2_prev = p2_next

    nc.compile()
    return nc


def _host_prep(style_encoding, content_in, dw_w, dw_b, pk_w, pk_b, pb_w, pb_b):
    """Shard + lay out inputs for the 8 cores (layout only, no math)."""
    f32 = np.float32
    bf = ml_dtypes.bfloat16
    common = {
        "dwT": np.ascontiguousarray(
            dw_w.reshape(8, 4, 128, 2, 2).transpose(2, 3, 4, 1, 0), f32),
        "dwb": np.ascontiguousarray(dw_b.reshape(8, 1), f32),
        "pbT": np.ascontiguousarray(
            pb_w.T.reshape(4, 128, 512).transpose(1, 0, 2), f32),
        "pbb": np.ascontiguousarray(pb_b.reshape(4, 128).T, f32),
        "pkwT": np.ascontiguousarray(
            pk_w.T.reshape(4, 128, 8).transpose(1, 0, 2), f32),
        "pkb": np.ascontiguousarray(pk_b.reshape(1, 8), f32),
    }
    ii = np.arange(128)
    common["mask16"] = (np.arange(16)[None, :] == (ii[:, None] // 8)).astype(bf)
    # post-shift row order: kh0 at rows 0:16, kh2 at 16:32, kh1 at 48:64
    w2 = np.zeros((128, 128), bf)
    for base in (0, 64):
        w2[base + 0 + ii // 8, ii] = 1     # kh = 0
        w2[base + 16 + ii // 8, ii] = 1    # kh = 2 (shifted into 16:32)
        w2[base + 48 + ii // 8, ii] = 1    # kh = 1 (shifted into 48:64)
    common["w2"] = w2
    common["repl8"] = (np.arange(8)[:, None] == (ii[None, :] % 8)).astype(f32)

    # padded pitch-68 bf16 content, all cores at once
    xp = np.pad(content_in, ((0, 0), (0, 0), (1, 1), (1, 1)), mode="reflect")
    buf = np.zeros((16, CH, NR, PW), f32)
    buf[:, :, :, 1:67] = xp
    xb = buf.reshape(16, 4, 128, EXT).astype(bf)

    in_maps = []
    for i in range(N_CORES):
        x16 = np.zeros((NSAMP, 4, 128, XAL), bf)
        x16[:, :, :, :EXT] = xb[NSAMP * i: NSAMP * (i + 1)]
        se = style_encoding[NSAMP * i: NSAMP * (i + 1)]
        in_maps.append({
            "x16": np.ascontiguousarray(x16.reshape(8, 128, XAL)),
            "style": np.ascontiguousarray(
                se.reshape(NSAMP, 4, 128, 16).transpose(2, 0, 1, 3), f32),
            **common,
        })
    return in_maps


def kernel(style_encoding, content_in, dw_w, dw_b, pk_w, pk_b, pb_w, pb_b):
    global LAST_RESULTS
    import os
    if "nc" not in _CACHE:
        _CACHE["nc"] = _build()
    nc = _CACHE["nc"]
    in_maps = _host_prep(style_encoding, content_in, dw_w, dw_b,
                         pk_w, pk_b, pb_w, pb_b)
    res = run_bass_kernel_spmd(
        nc, in_maps, core_ids=list(range(N_CORES)),
        trace=bool(os.environ.get("ADACONV_TRACE")))
    LAST_RESULTS = res
    outs = []
    for i in range(N_CORES):
        o = np.asarray(res.results[i]["out"]).astype(np.float32)
        outs.append(o.reshape(NSAMP, 4, 128, 64, 64).reshape(NSAMP, CH, 64, 64))
    return np.concatenate(outs, axis=0)


# revision 19
# speedup vs baseline: 5.2184x; 1.1338x over previous
"""AdaConv kernel for 8 TRN2 NeuronCores — data-parallel over batch.

Two-stage stencil formulation. Math identical to the reference after
collapsing the per-sample grouped convs:
    D[n,g,h,w] = sum_{j,kh,kw} d[n,j,kh,kw] * xpad[n,8g+j,h+kh,w+kw]
    out = leaky(S[n]*D[n,c//8] + bias[n,c]) * (x - mean)/std

Per core (2 samples, 8 tiles of 128 channels; x stored bf16, pitch-68 rows):
  stage 1 (PE): 3 accumulating matmuls (kw taps via rhs column offsets),
      M = 96 = (kh,g) pairs 32-aligned -> P2[(kh,g), p] in PSUM.
  fold: PSUM->SBUF copy drops the seam cols (dense 64-pitch bf16), then two
      SBUF->SBUF DMAs shift the kh=1,2 slabs by kh*64 so taps align.
  stage 2 (PE): ONE K=48 matmul per 7-row slot -> D replicated over the 8
      channels of each group, directly in [128, px] dense layout.
  ScalarE evicts with fused leaky(S*rstd*D + bias*rstd) (valid: r>0).
  Stats on DVE: bn_stats/bn_aggr (bf16 x); rstd via Newton rsqrt on DVE
      (no ScalarE Sqrt -> no activation-table thrash).
  Final: DVE tensor_scalar xn = x - mean, tensor_tensor out = xn * predn.
"""

import numpy as np
import ml_dtypes
from contextlib import ExitStack

import concourse.bass as bass
import concourse.tile as tile
from concourse import bacc, mybir
from concourse.bass_utils import run_bass_kernel_spmd

F32 = mybir.dt.float32
I32 = mybir.dt.int32
BF16 = mybir.dt.bfloat16
AF = mybir.ActivationFunctionType
ALU = mybir.AluOpType
AX = mybir.AxisListType

N_CORES = 8
NSAMP = 2           # samples per core
CH = 512
H = W = 64
PW = 68             # row pitch (junk col 0, padded cols 1..66, junk col 67)
NR = 66             # padded rows
EXT = PW * NR       # 4488 flat extent
XAL = EXT + 8       # 4496 allocated (stencil over-read + even)
DE = 64 * NR        # 4224 dense P2 extent
RSQRT_MAGIC = np.int32(0x5F3759DF).view(np.float32).item()

LAST_RESULTS = None  # BassKernelResults of the most recent run (for test.py)
_CACHE = {}


def _build():
    nc = bacc.Bacc("TRN2", target_bir_lowering=False, debug=False)

    x16_d = nc.dram_tensor("x16", [8, 128, XAL], BF16, kind="ExternalInput")
    style_d = nc.dram_tensor("style", [128, NSAMP, 4, 16], F32, kind="ExternalInput")
    dwT_d = nc.dram_tensor("dwT", [128, 2, 2, 4, 8], F32, kind="ExternalInput")
    dwb_d = nc.dram_tensor("dwb", [8, 1], F32, kind="ExternalInput")
    pbT_d = nc.dram_tensor("pbT", [128, 4, 512], F32, kind="ExternalInput")
    pbb_d = nc.dram_tensor("pbb", [128, 4], F32, kind="ExternalInput")
    pkwT_d = nc.dram_tensor("pkwT", [128, 4, 8], F32, kind="ExternalInput")
    pkb_d = nc.dram_tensor("pkb", [1, 8], F32, kind="ExternalInput")
    mask16_d = nc.dram_tensor("mask16", [128, 16], BF16, kind="ExternalInput")
    w2_d = nc.dram_tensor("w2", [128, 128], BF16, kind="ExternalInput")
    repl8_d = nc.dram_tensor("repl8", [8, 128], F32, kind="ExternalInput")
    out_d = nc.dram_tensor("out", [8, 128, H * W], BF16, kind="ExternalOutput")

    with tile.TileContext(nc) as tc, ExitStack() as ctx:
        const = ctx.enter_context(tc.tile_pool(name="const", bufs=1))
        small = ctx.enter_context(tc.tile_pool(name="small", bufs=1))
        x16p = ctx.enter_context(tc.tile_pool(name="x16", bufs=8))
        p2p = ctx.enter_context(tc.tile_pool(name="p2", bufs=2))
        statp = ctx.enter_context(tc.tile_pool(name="stat", bufs=2))
        xnp = ctx.enter_context(tc.tile_pool(name="xn", bufs=2))
        prednp = ctx.enter_context(tc.tile_pool(name="pred", bufs=2))
        outp = ctx.enter_context(tc.tile_pool(name="outp", bufs=2))
        psumA = ctx.enter_context(
            tc.tile_pool(name="psumA", bufs=2, space="PSUM"))
        psumB = ctx.enter_context(
            tc.tile_pool(name="psumB", bufs=2, space="PSUM"))

        # ---- params (small, first on the sync queue) ----
        style_sb = const.tile([128, NSAMP, 4, 16], F32)
        nc.sync.dma_start(style_sb[:], style_d[:])
        dwT_sb = const.tile([128, 2, 2, 4, 8], F32)
        nc.sync.dma_start(dwT_sb[:], dwT_d[:])
        dwb_sb = const.tile([8, 1], F32)
        nc.sync.dma_start(dwb_sb[:], dwb_d[:])
        repl8_sb = const.tile([8, 128], F32)
        nc.sync.dma_start(repl8_sb[:], repl8_d[:])
        mask16_sb = const.tile([128, 16], BF16)
        nc.sync.dma_start(mask16_sb[:], mask16_d[:])
        w2_sb = const.tile([128, 128], BF16)
        nc.sync.dma_start(w2_sb[:], w2_d[:])
        pkb_sb = const.tile([1, 8], F32)
        nc.scalar.dma_start(pkb_sb[:], pkb_d[:])
        pbb_sb = const.tile([128, 4], F32)
        nc.scalar.dma_start(pbb_sb[:], pbb_d[:])
        pbT_sb = const.tile([128, 4, 512], F32)
        nc.scalar.dma_start(pbT_sb[:], pbT_d[:])
        pkwT_sb = const.tile([128, 4, 8], F32)
        nc.scalar.dma_start(pkwT_sb[:], pkwT_d[:])

        # content loads for the first tiles (prefetch window of 3)
        x16s = []
        for _ in range(8):
            x16 = x16p.tile([128, XAL], BF16, tag="x16")
            x16s.append(x16)

        def emit_xin(ts):
            for c in range(4):
                lo, hi = c * 1124, (c + 1) * 1124
                nc.sync.dma_start(x16s[ts][:, lo:hi], x16_d[ts][:, lo:hi])

        for ts in range(4):
            emit_xin(ts)

        # ---- prologue: kernel-predictor math (tiny, f32) ----
        W1_sb = const.tile([128, NSAMP, 3, 64], BF16)     # stage-1 weights
        bias_sb = const.tile([128, 4, NSAMP], F32)        # per-channel bias
        Sb_sb = const.tile([128, NSAMP], F32)             # S[n] on 128 parts
        d_sb = small.tile([8, NSAMP, 9], F32)
        dcol_sb = small.tile([128, NSAMP, 9], F32)
        ssum_sb = small.tile([128, 4, NSAMP], F32)
        pkwsum_sb = small.tile([128, 4], F32)
        pkbsum_sb = small.tile([1, 1], F32)
        S_sb = small.tile([1, NSAMP], F32)
        magic_sb = const.tile([128, 1], F32)
        nc.vector.memset(magic_sb[:], RSQRT_MAGIC)
        nc.vector.memset(W1_sb[:], 0.0)

        nc.vector.tensor_reduce(pkbsum_sb[:], pkb_sb[:], axis=AX.X, op=ALU.add)
        for kt in range(4):
            nc.vector.tensor_reduce(
                pkwsum_sb[:, kt:kt + 1], pkwT_sb[:, kt, :], axis=AX.X, op=ALU.add)

        for s in range(NSAMP):
            # d = leaky(conv2x2(style, dw_w) + dw_b):  16 accumulating matmuls
            psA0 = psumA.tile([128, 1024], F32, tag="psA")
            ps_d = psA0[0:8, 0:9]
            i = 0
            for ky in range(2):
                for kx in range(2):
                    for kt in range(4):
                        rhs = style_sb[:, s, kt, :].rearrange(
                            "p (y x) -> p y x", x=4)[:, ky:ky + 3, kx:kx + 3]
                        nc.tensor.matmul(
                            ps_d, dwT_sb[:, ky, kx, kt, :], rhs,
                            start=(i == 0), stop=(i == 15))
                        i += 1
            nc.scalar.activation(
                d_sb[:, s, :], ps_d, AF.Lrelu, bias=dwb_sb[:], alpha=0.01)

            # replicate d over channels: dcol[c,t] = d[c%8,t]
            psA1 = psumA.tile([128, 1024], F32, tag="psA")
            ps_dc = psA1[:, 0:9]
            nc.tensor.matmul(ps_dc, repl8_sb[:], d_sb[:, s, :])
            nc.vector.tensor_copy(dcol_sb[:, s, :], ps_dc)

            # stage-1 weights W1[kw][ch, kh*16+g] = d[ch%8, kh, kw]*(g==ch//8)
            for kh in range(3):
                for kw in range(3):
                    nc.vector.tensor_scalar(
                        W1_sb[:, s, kw, kh * 16: kh * 16 + 16], mask16_sb[:],
                        dcol_sb[:, s, 3 * kh + kw: 3 * kh + kw + 1], None,
                        ALU.mult)

            # style spatial sums (s_d * 16)
            for kt in range(4):
                nc.vector.tensor_reduce(
                    ssum_sb[:, kt, s:s + 1], style_sb[:, s, kt, :],
                    axis=AX.X, op=ALU.add)

        # bias[c] = s_d @ pb_w[c] + pb_b[c]   (both samples batched)
        for mt in range(4):
            psB0 = psumB.tile([128, 1024], F32, tag="psB")
            ps_b = psB0[:, 0:NSAMP]
            for kt in range(4):
                nc.tensor.matmul(
                    ps_b, pbT_sb[:, kt, mt * 128:(mt + 1) * 128],
                    ssum_sb[:, kt, :], start=(kt == 0), stop=(kt == 3))
            nc.vector.tensor_scalar(
                bias_sb[:, mt, :], ps_b, 1.0 / 16.0,
                pbb_sb[:, mt:mt + 1], ALU.mult, ALU.add)

        # S = s_d @ pkw_sum + sum(pk_b)
        psB1 = psumB.tile([128, 1024], F32, tag="psB")
        ps_S = psB1[0:1, 0:NSAMP]
        for kt in range(4):
            nc.tensor.matmul(
                ps_S, pkwsum_sb[:, kt:kt + 1], ssum_sb[:, kt, :],
                start=(kt == 0), stop=(kt == 3))
        nc.vector.tensor_scalar(
            S_sb[:], ps_S, 1.0 / 16.0, pkbsum_sb[:], ALU.mult, ALU.add)
        nc.gpsimd.partition_broadcast(Sb_sb[:], S_sb[:])

        # ---- per-tile state ----
        mv_all = small.tile([128, 8, 2], F32)        # (mean, var) per tile
        v_all = small.tile([128, 8], F32)            # var + eps
        y_all = small.tile([128, 8], F32)            # rsqrt iterate
        t_all = small.tile([128, 8], F32)
        scaleS_all = small.tile([128, 8], F32)       # S * rstd
        biasS_all = small.tile([128, 8], F32)        # bias * rstd

        def bn_stats_raw(out, in_):
            # bass's bn_stats wrapper mis-asserts the out shape for 3D
            # inputs; the HW op always writes 6 elements/partition.
            eng = nc.vector
            return eng.add_instruction(mybir.InstBNStats(
                name=eng.bass.get_next_instruction_name(),
                ins=[eng.lower_ap(in_)], outs=[eng.lower_ap(out)]))

        def emit_stats(ts):
            xr = x16s[ts][:, :EXT].rearrange("p (r w) -> p r w", w=PW)
            st = statp.tile([128, 8, 6], F32, tag="bn")
            for i in range(8):
                bn_stats_raw(st[:, i, :], xr[:, 1 + 8 * i: 9 + 8 * i, 2:66])
            nc.vector.bn_aggr(mv_all[:, ts, :], st[:])

        def emit_finalize(p):
            # pair-batched: tiles 2p, 2p+1
            sl = slice(2 * p, 2 * p + 2)
            s = (2 * p) // 4
            # v = var*4096/4095 + eps
            nc.vector.tensor_scalar(
                v_all[:, sl], mv_all[:, sl, 1], 4096.0 / 4095.0, 1e-5,
                ALU.mult, ALU.add)
            # Newton rsqrt: y0 from the bit trick, then 2 iterations
            nc.vector.tensor_scalar(
                t_all[:, sl].bitcast(I32), v_all[:, sl].bitcast(I32), 1,
                None, ALU.arith_shift_right)
            nc.vector.tensor_tensor(
                y_all[:, sl].bitcast(I32),
                magic_sb[:].bitcast(I32).to_broadcast([128, 2]),
                t_all[:, sl].bitcast(I32), ALU.subtract)
            y, t, v = y_all[:, sl], t_all[:, sl], v_all[:, sl]
            for _ in range(2):
                nc.vector.tensor_tensor(t, y, y, ALU.mult)
                nc.vector.tensor_tensor(t, t, v, ALU.mult)
                nc.vector.tensor_scalar(t, t, -0.5, 1.5, ALU.mult, ALU.add)
                nc.vector.tensor_tensor(y, y, t, ALU.mult)
            nc.vector.tensor_scalar(
                scaleS_all[:, sl], y, Sb_sb[:, s:s + 1], None, ALU.mult)
            kt0 = (2 * p) % 4
            nc.vector.tensor_tensor(
                biasS_all[:, sl], y, bias_sb[:, kt0:kt0 + 2, s], ALU.mult)

        def emit_stage1_pair(p):
            # tiles a = 2p (psum cols 0:48), b = 2p+1 (psum cols 64:112)
            a, b = 2 * p, 2 * p + 1
            s = a // 4
            xa, xb = x16s[a], x16s[b]
            p2 = p2p.tile([128, DE], BF16, tag="p2")
            r0 = 0
            while r0 < NR:
                nr0 = min(7, NR - r0)
                nr1 = min(7, NR - r0 - nr0)
                rows = [nr0] + ([nr1] if nr1 > 0 else [])
                psA = psumA.tile([128, 1024], F32, tag="psA")
                for kw in range(3):
                    for si, nr in enumerate(rows):
                        rr = r0 + (rows[0] if si else 0)
                        lo = rr * PW + kw
                        cw = nr * PW
                        nc.tensor.matmul(
                            psA[0:64, si * 512: si * 512 + cw],
                            W1_sb[:, s, kw, :], xa[:, lo: lo + cw],
                            start=(kw == 0), stop=(kw == 2),
                            skip_group_check=True)
                        nc.tensor.matmul(
                            psA[64:128, si * 512: si * 512 + cw],
                            W1_sb[:, s, kw, :], xb[:, lo: lo + cw],
                            start=(kw == 0), stop=(kw == 2),
                            skip_group_check=True)
                nrt = sum(rows)
                # one seam-dropping copy evacuates BOTH tiles
                if len(rows) == 2 and rows[0] == rows[1]:
                    srcv = psA[0:112, :].rearrange(
                        "p (u q) -> p u q", q=512)[:, :, :rows[0] * PW]                         .rearrange("p u (r w) -> p u r w", w=PW)[:, :, :, 1:65]
                    nc.scalar.copy(p2[0:112, r0 * 64: (r0 + nrt) * 64], srcv)
                else:
                    off = 0
                    for si, nr in enumerate(rows):
                        srcv = psA[0:112, si * 512: si * 512 + nr * PW]                             .rearrange("p (r w) -> p r w", w=PW)[:, :, 1:65]
                        nc.scalar.copy(
                            p2[0:112, (r0 + off) * 64: (r0 + off + nr) * 64],
                            srcv)
                        off += nr
                r0 += nrt
            # slab shifts into the gap rows (kh1 -> rows 48:64, kh2 -> 16:32),
            # split into column halves so stage-2 deps resolve progressively
            HC = 2112
            for lo2, hi2 in ((0, HC), (HC, DE)):
                for base in (0, 64):
                    nc.gpsimd.dma_start(
                        p2[base + 48: base + 64, lo2: min(hi2, DE - 64)],
                        p2[base + 16: base + 32,
                           lo2 + 64: min(hi2, DE - 64) + 64])
                    nc.gpsimd.dma_start(
                        p2[base + 16: base + 32, lo2: min(hi2, DE - 128)],
                        p2[base + 32: base + 48,
                           lo2 + 128: min(hi2, DE - 128) + 128])
            return p2

        def emit_stage2(ts, p2):
            half64 = 64 * (ts % 2)      # a -> rows 0:64, b -> rows 64:128
            x16 = x16s[ts]
            predn = prednp.tile([128, H * W], BF16, tag="pred")
            xn = xnp.tile([128, XAL], BF16, tag="xn")
            out_sb = outp.tile([128, H * W], BF16, tag="out")
            nc.vector.tensor_scalar(
                xn[:], x16[:], mv_all[:, ts, 0:1], None, ALU.subtract)
            xnr = xn[:, :EXT].rearrange("p (r w) -> p r w", w=PW)
            for half in range(2):
                for rt in (0, 16):
                    r0 = half * 32 + rt
                    psB = psumB.tile([128, 1024], F32, tag="psB")
                    for sub in (0, 512):
                        rr = r0 + (8 if sub else 0)
                        nc.tensor.matmul(
                            psB[:, sub:sub + 512],
                            w2_sb[half64: half64 + 64, :],
                            p2[half64: half64 + 64, rr * 64: (rr + 8) * 64],
                            start=True, stop=True)
                    nc.scalar.activation(
                        predn[:, r0 * 64: (r0 + 16) * 64], psB[:, :],
                        AF.Lrelu, bias=biasS_all[:, ts:ts + 1],
                        scale=scaleS_all[:, ts:ts + 1], alpha=0.01)
                lo, hi = half * 2048, (half + 1) * 2048
                r0 = half * 32
                nc.vector.tensor_tensor(
                    out_sb[:, lo:hi].rearrange("p (r w) -> p r w", w=64),
                    xnr[:, 1 + r0: 33 + r0, 2:66],
                    predn[:, lo:hi].rearrange("p (r w) -> p r w", w=64),
                    ALU.mult)
                nc.sync.dma_start(out_d[ts][:, lo:hi], out_sb[:, lo:hi])

        # ---- software-pipelined main loop over tile pairs ----
        emit_stats(0)
        emit_stats(1)
        emit_finalize(0)
        p2_prev = emit_stage1_pair(0)
        for p in range(4):
            if p + 1 < 4:
                for ts in (2 * p + 4, 2 * p + 5):
                    if ts < 8:
                        emit_xin(ts)
                emit_stats(2 * p + 2)
                emit_stats(2 * p + 3)
                emit_finalize(p + 1)
                p2_next = emit_stage1_pair(p + 1)
            emit_stage2(2 * p, p2_prev)
            emit_stage2(2 * p + 1, p2_prev)
            if p + 1 < 4:
                p2_prev = p2_next

    nc.compile()
    return nc


def _host_prep(style_encoding, content_in, dw_w, dw_b, pk_w, pk_b, pb_w, pb_b):
    """Shard + lay out inputs for the 8 cores (layout only, no math)."""
    f32 = np.float32
    bf = ml_dtypes.bfloat16
    common = {
        "dwT": np.ascontiguousarray(
            dw_w.reshape(8, 4, 128, 2, 2).transpose(2, 3, 4, 1, 0), f32),
        "dwb": np.ascontiguousarray(dw_b.reshape(8, 1), f32),
        "pbT": np.ascontiguousarray(
            pb_w.T.reshape(4, 128, 512).transpose(1, 0, 2), f32),
        "pbb": np.ascontiguousarray(pb_b.reshape(4, 128).T, f32),
        "pkwT": np.ascontiguousarray(
            pk_w.T.reshape(4, 128, 8).transpose(1, 0, 2), f32),
        "pkb": np.ascontiguousarray(pk_b.reshape(1, 8), f32),
    }
    ii = np.arange(128)
    common["mask16"] = (np.arange(16)[None, :] == (ii[:, None] // 8)).astype(bf)
    # post-shift row order: kh0 at rows 0:16, kh2 at 16:32, kh1 at 48:64
    w2 = np.zeros((128, 128), bf)
    for base in (0, 64):
        w2[base + 0 + ii // 8, ii] = 1     # kh = 0
        w2[base + 16 + ii // 8, ii] = 1    # kh = 2 (shifted into 16:32)
        w2[base + 48 + ii // 8, ii] = 1    # kh = 1 (shifted into 48:64)
    common["w2"] = w2
    common["repl8"] = (np.arange(8)[:, None] == (ii[None, :] % 8)).astype(f32)

    # padded pitch-68 bf16 content, all cores at once
    xp = np.pad(content_in, ((0, 0), (0, 0), (1, 1), (1, 1)), mode="reflect")
    buf = np.zeros((16, CH, NR, PW), f32)
    buf[:, :, :, 1:67] = xp
    xb = buf.reshape(16, 4, 128, EXT).astype(bf)

    in_maps = []
    for i in range(N_CORES):
        x16 = np.zeros((NSAMP, 4, 128, XAL), bf)
        x16[:, :, :, :EXT] = xb[NSAMP * i: NSAMP * (i + 1)]
        se = style_encoding[NSAMP * i: NSAMP * (i + 1)]
        in_maps.append({
            "x16": np.ascontiguousarray(x16.reshape(8, 128, XAL)),
            "style": np.ascontiguousarray(
                se.reshape(NSAMP, 4, 128, 16).transpose(2, 0, 1, 3), f32),
            **common,
        })
    return in_maps


def kernel(style_encoding, content_in, dw_w, dw_b, pk_w, pk_b, pb_w, pb_b):
    global LAST_RESULTS
    import os
    if "nc" not in _CACHE:
        _CACHE["nc"] = _build()
    nc = _CACHE["nc"]
    in_maps = _host_prep(style_encoding, content_in, dw_w, dw_b,
                         pk_w, pk_b, pb_w, pb_b)
    res = run_bass_kernel_spmd(
        nc, in_maps, core_ids=list(range(N_CORES)),
        trace=bool(os.environ.get("ADACONV_TRACE")))
    LAST_RESULTS = res
    outs = []
    for i in range(N_CORES):
        o = np.asarray(res.results[i]["out"]).astype(np.float32)
        outs.append(o.reshape(NSAMP, 4, 128, 64, 64).reshape(NSAMP, CH, 64, 64))
    return np.concatenate(outs, axis=0)
